# revision 26
# baseline (speedup 1.0000x reference)
"""MultiHeadAttention Trainium2 kernel, 8-way tensor-parallel by head.

Problem: B=4, S=2048, D=1024, 16 heads, d_k=64 (nn_MultiHeadAttention_67585605370071).

Sharding: each core owns 2 heads (128 of the 1024 hidden dims):
  Wq/Wk/Wv column-sharded by head, Wo row-sharded; the 8 partial outputs
  are summed on the host (the row-shard reduction) and bo is added there.

Dataflow per core (matmuls bf16 / fp8-DoubleRow, f32 PSUM accumulation):
  - host passes x pre-transposed twice: xT bf16 [1024, 8192] (V path) and
    xT fp8 [1024, 8192] (Q/K path)
  - Q/K projections run fp8 DoubleRow (weights hosted as fp8(32*W), biases
    pre-scaled by 32; the extra 32*32 factor is folded into the exp scale)
  - V projection runs bf16; V rotated to token-major layout via SBUF->SBUF
    DMA transposes, then quantized fp8 into the augmented layout
    [V_A | 1 | pad | V_B | 1 | pad] by the GpSimd engine; the ones column
    makes the AV matmul emit softmax denominators as row 64 of its PSUM tile
  - scoresT = Kt.T @ Qt per k-tile pair; exp is split between the ScalarE
    ACT (table exp) and a custom DVE microcode op EXP16_POLY_ANT
    ((1 + ks + (ks)^2/2)^16, 8/8 uop stages) so neither engine saturates;
    both emit fp8
  - AV via fp8 DoubleRow matmuls (2 k-tiles per pass), one chunk behind the
    scores stream so the PE never stalls on exp latency
  - normalize via DVE reciprocal_approx_fast + GpSimd partition_broadcast +
    DVE tensor_mul
  - out-proj per 128-token tile; even tiles are copied PSUM->SBUF bf16
    (split between ACT-copy and DVE) and DMA'd, odd tiles DMA straight from
    PSUM to DRAM in f32; projection bias-adds run as ACT Identity+bias
  - batches are software-pipelined: projection blocks for batch b+1 and the
    previous unit's out-proj are drained between score chunks
"""
import numpy as np
import ml_dtypes

import concourse.bass as bass
import concourse.bacc as bacc
import concourse.mybir as mybir
import concourse.tile as tile
from concourse.bass_utils import run_bass_kernel_spmd

import concourse.dve_ops as dve_ops_mod
from concourse.dve_spec import C0, C1, One, Spec, Src0, lower as dve_lower, sq
from concourse.dve_uop import DveOpSpec

BF16 = mybir.dt.bfloat16
F32 = mybir.dt.float32
F8 = mybir.dt.float8e4
bf16 = ml_dtypes.bfloat16
f8 = ml_dtypes.float8_e4m3

B, S, D = 4, 2048, 1024
NT = B * S            # 8192 tokens
N_CORES = 8
KT_PER_B = S // 128   # 16 k-tiles per batch
QB_PER_B = S // 512   # 4 query blocks per batch
TB_PER_B = S // 512   # 4 token blocks per batch (projection)
NCH = KT_PER_B // 2   # 8 score/exp chunks per attention unit

# fp8-e4m3 attention-weights path: expT and V quantized to fp8 so the AV
# matmul can use DoubleRow (2x PE throughput).
VG = 144              # vaug per-k-tile column group
HOFF = 72             # head B column offset inside a group

# Q/K projections in fp8 DoubleRow; weights scaled by 32 on the host, the
# 32*32 factor folded into the exp scale.
QK_DR = False
QK_W_SCALE = 32.0
EXP_SCALE = 0.125 / (QK_W_SCALE * QK_W_SCALE if QK_DR else 1.0)

# exp tiles handed to the DVE custom op instead of ScalarE, per unit
DVE_EXP = {(c, 1) for c in range(7)}
SPLIT_EXP = True     # exp per [128,512] half (4 sp bufs) vs full [128,1024]
SPLIT_OUTPROJ = False  # out-proj mm/copy as separate bg closures
NDRAIN = 3           # bg closures drained per chunk
SP_BUFS = 4          # score-PSUM buffers ([128,512] each)
SH_BUFS = 2          # shared pp/po PSUM buffers

_COMPILED = None

# ---------------------------------------------------------------------------
# Custom DVE exp op: out = exp(in * scale) via (1 + ks + (ks)^2/2)^16,
# k = scale/16 — quadratic (4 ALU stages) + 4 squarings = 8/8 v3 stages.
# Rel err vs true exp: rms ~1e-4; after fp8 output quantization 99.96% of
# outputs are bit-identical to fp8(exp(x)).
EXP16_NAME = "EXP16_POLY_ANT"


def _exp16_consts(scale):
    k = scale / 16.0
    return (k * k / 2.0, k)


def _exp16_ref(in0, in1, s0, s1, imm2):
    b = ((in0.astype(np.float32) * s0 + s1) * in0 + 1.0).astype(np.float32)
    for _ in range(4):
        b = (b * b).astype(np.float32)
    return b


def _register_exp16():
    for op in dve_ops_mod.OPS:
        if op.name == EXP16_NAME:
            return op
    body = sq(sq(sq(sq((Src0 * C0 + C1) * Src0 + One))))
    spec = Spec(body=body, reference=_exp16_ref)
    row = max(dve_ops_mod._SUB_OPCODE_FOR_NAME.values()) + 1
    assert row < 0x20, "custom-DVE opcode rows exhausted"
    dve_ops_mod._SUB_OPCODE_FOR_NAME[EXP16_NAME] = row
    shas = {}
    for ver in ("v3", "v4"):
        s = DveOpSpec(name=EXP16_NAME, opcode=row,
                      uops=dve_lower(spec, ver=ver), rd1_en=False)
        shas[ver] = s.sha(ver)
    op = dve_ops_mod.DveOp(EXP16_NAME, spec, False, shas)
    dve_ops_mod.OPS.append(op)
    dve_ops_mod.CUSTOM_DVE_SPECS[EXP16_NAME] = spec
    return op


# ---------------------------------------------------------------------------
def _build():
    exp_op = _register_exp16()
    exp_s0, exp_s1 = _exp16_consts(EXP_SCALE)
    Ident = mybir.ActivationFunctionType.Identity
    CopyF = mybir.ActivationFunctionType.Copy

    nc = bacc.Bacc("TRN2", target_bir_lowering=False, debug=False,
                   num_devices=N_CORES)

    xt_p = nc.declare_dram_parameter("xt", [D, NT], BF16, isOutput=False)
    wv_p = nc.declare_dram_parameter("wv", [128, D], BF16, isOutput=False)
    wo_p = nc.declare_dram_parameter("wo", [128, D], BF16, isOutput=False)
    bq_p = nc.declare_dram_parameter("bq", [128, 1], F32, isOutput=False)
    bk_p = nc.declare_dram_parameter("bk", [128, 1], F32, isOutput=False)
    bv_p = nc.declare_dram_parameter("bv", [128, 1], F32, isOutput=False)
    if QK_DR:
        xt8_p = nc.declare_dram_parameter("xt8", [D, NT], F8, isOutput=False)
        wq_p = nc.declare_dram_parameter("wq", [128, D], F8, isOutput=False)
        wk_p = nc.declare_dram_parameter("wk", [128, D], F8, isOutput=False)
    else:
        wq_p = nc.declare_dram_parameter("wq", [128, D], BF16, isOutput=False)
        wk_p = nc.declare_dram_parameter("wk", [128, D], BF16, isOutput=False)
    out16_p = nc.declare_dram_parameter("out16", [NT, D], BF16, isOutput=True)

    WDT = F8 if QK_DR else BF16

    with tile.TileContext(nc) as tc:
        with (
            tc.tile_pool(name="consts", bufs=1) as consts,
            tc.tile_pool(name="xts", bufs=3) as xts_pool,
            tc.tile_pool(name="qkv", bufs=2) as qkv_pool,
            tc.tile_pool(name="vtrp", bufs=4) as vtr_pool,
            tc.tile_pool(name="expp", bufs=4) as exp_pool,
            tc.tile_pool(name="attnp", bufs=3) as attn_pool,
            tc.tile_pool(name="small", bufs=3) as small,
            tc.tile_pool(name="outp", bufs=3) as out_pool,
            tc.tile_pool(name="ps_sp", bufs=2, space=bass.MemorySpace.PSUM) as ps_sp,
            tc.tile_pool(name="ps_av", bufs=2, space=bass.MemorySpace.PSUM) as ps_av,
            tc.tile_pool(name="ps_pp", bufs=1, space=bass.MemorySpace.PSUM) as ps_pp,
            tc.tile_pool(name="ps_po", bufs=1, space=bass.MemorySpace.PSUM) as ps_po,
        ):
            wq = consts.tile([128, D], WDT, name="wq")
            wk = consts.tile([128, D], WDT, name="wk")
            wv = consts.tile([128, D], BF16, name="wv")
            wo = consts.tile([128, D], BF16, name="wo")
            bq = consts.tile([128, 1], F32, name="bq")
            bk = consts.tile([128, 1], F32, name="bk")
            bv = consts.tile([128, 1], F32, name="bv")
            nc.sync.dma_start(wq[:], wq_p[:])
            nc.sync.dma_start(wk[:], wk_p[:])
            nc.sync.dma_start(wv[:], wv_p[:])
            nc.sync.dma_start(wo[:], wo_p[:])
            nc.sync.dma_start(bq[:], bq_p[:])
            nc.sync.dma_start(bk[:], bk_p[:])
            nc.sync.dma_start(bv[:], bv_p[:])

            # per-batch, per-block persistent tiles, allocated lazily
            qt = {}          # qt[b][tb] -> [128, 512]
            kt = {}          # kt[b][tb] -> [128, 512] (4 k-tiles each)
            vts = {}         # vts[b][tb] -> [128, 512]
            vaug = {}        # vaug[b][tb] -> [128, 4*VG] fp8
            xts_t = {}       # prefetched xT stacks (bf16, fp8)

            def _alloc_batch(b):
                qt[b] = [qkv_pool.tile([128, 512], BF16, tag=f"qt{t}",
                                       name=f"qt{b}_{t}") for t in range(4)]
                kt[b] = [qkv_pool.tile([128, 512], BF16, tag=f"kt{t}",
                                       name=f"kt{b}_{t}") for t in range(4)]
                vts[b] = [qkv_pool.tile([128, 512], BF16, tag=f"vts{t}",
                                        name=f"vts{b}_{t}") for t in range(4)]
                vaug[b] = []
                for t in range(4):
                    va = qkv_pool.tile([128, 4 * VG], F8, tag=f"vaug{t}",
                                       name=f"vaug{b}_{t}")
                    va3 = va.rearrange("p (k c) -> p k c", c=VG)
                    nc.gpsimd.memset(va3[:, :, 64:65], 1.0)
                    nc.gpsimd.memset(va3[:, :, HOFF + 64:HOFF + 65], 1.0)
                    vaug[b].append(va)

            def emit_proj_dma(b, tb):
                """Prefetch the 512-token xT stack(s) for block (b, tb)."""
                if b not in qt:
                    _alloc_batch(b)
                tok0 = b * S + tb * 512
                xts = xts_pool.tile([128, 8 * 512], BF16, tag="xts",
                                    name=f"xts{b}_{tb}")
                src3 = xt_p.rearrange("(kd p) t -> p kd t", p=128)
                dst3 = xts.rearrange("p (kd t) -> p kd t", t=512)
                nc.sync.dma_start(dst3[:, :, :], src3[:, :, tok0:tok0 + 512])
                if QK_DR:
                    xts8 = xts_pool.tile([128, 8 * 512], F8, tag="xts8",
                                         name=f"xts8{b}_{tb}")
                    src83 = xt8_p.rearrange("(kd p) t -> p kd t", p=128)
                    dst83 = xts8.rearrange("p (kd t) -> p kd t", t=512)
                    nc.sync.dma_start(dst83[:, :, :], src83[:, :, tok0:tok0 + 512])
                    xts_t[(b, tb)] = (xts, xts8)
                else:
                    xts_t[(b, tb)] = (xts, xts)

            def emit_proj_compute(b, tb):
                for clo in proj_closures(b, tb):
                    clo()

            def proj_closures(b, tb):
                """The projection block as a list of small closures so its PE
                work can be interleaved between attention score chunks.
                V runs first so its rotation DMAs are in flight long before
                the Pool quantize copies (emitted last) need them."""
                xts, xts8 = xts_t.pop((b, tb))

                # Build producer (PE matmul) and consumer (ACT bias / Pool
                # quantize) closures, then interleave so every consumer
                # drains a few slots after its producer — an in-order
                # engine never head-of-line blocks on a PE matmul.
                ppk = ps_pp.tile([128, 512], F32, tag="pp", name=f"ppk{b}_{tb}")
                ppq = ps_pp.tile([128, 512], F32, tag="pp", name=f"ppq{b}_{tb}")
                ppv = ps_pp.tile([128, 512], F32, tag="pp", name=f"ppv{b}_{tb}")

                def mk_qk(pp, w_sb):
                    if QK_DR:
                        w3 = w_sb.rearrange("p (k m) -> p k m", m=128)
                        x3 = xts8.rearrange("p (k t) -> p k t", t=512)

                        def mmdr(p0):
                            for p in (p0, p0 + 1):
                                nc.tensor.matmul(
                                    pp[:], w3[:, 2 * p:2 * p + 2, :],
                                    x3[:, 2 * p:2 * p + 2, :],
                                    start=(p == 0), stop=(p == 3),
                                    perf_mode=mybir.MatmulPerfMode.DoubleRow)
                        return [lambda p0=p0: mmdr(p0) for p0 in (0, 2)]

                    def mm2(kd0):
                        for kd in (kd0, kd0 + 1):
                            nc.tensor.matmul(
                                pp[:], w_sb[:, 128 * kd:128 * kd + 128],
                                xts[:, 512 * kd:512 * kd + 512],
                                start=(kd == 0), stop=(kd == 7))
                    return [lambda kd0=kd0: mm2(kd0) for kd0 in (0, 2, 4, 6)]

                def mm2v(kd0):
                    for kd in (kd0, kd0 + 1):
                        nc.tensor.matmul(
                            ppv[:], wv[:, 128 * kd:128 * kd + 128],
                            xts[:, 512 * kd:512 * kd + 512],
                            start=(kd == 0), stop=(kd == 7))

                vtrs = [vtr_pool.tile([128, 128], BF16, tag=f"vtr{ti}",
                                      name=f"vtr{b}_{tb}_{ti}")
                        for ti in range(4)]

                def rot_dma():
                    for ti in range(4):
                        nc.sync.dma_start_transpose(
                            vtrs[ti][:], vts[b][tb][:, 128 * ti:128 * ti + 128])

                va4 = vaug[b][tb].rearrange("p (k g e) -> p k g e", g=2, e=HOFF)

                def rot_q(ti):
                    vtr3 = vtrs[ti].rearrange("p (g e) -> p g e", e=64)
                    nc.gpsimd.tensor_copy(va4[:, ti, :, 0:64], vtr3[:, :, :])

                kbias = lambda: nc.scalar.activation(kt[b][tb][:], ppk[:],
                                                     Ident, bias=bk[:])
                qbias = lambda: nc.scalar.activation(qt[b][tb][:], ppq[:],
                                                     Ident, bias=bq[:])
                vbias = lambda: nc.scalar.activation(vts[b][tb][:], ppv[:],
                                                     Ident, bias=bv[:])
                kmm = mk_qk(ppk, wk)
                qmm = mk_qk(ppq, wq)
                vmm = [lambda kd0=kd0: mm2v(kd0) for kd0 in (0, 2, 4, 6)]
                return (kmm + [kbias] + qmm + [qbias] + vmm
                        + [vbias, rot_dma]
                        + [lambda ti=ti: rot_q(ti) for ti in range(4)])

            def outproj_closures(b, qb, attn):
                """Interleaved [mm, mm, copy, mm, copy, ...] so each
                PSUM->SBUF copy drains ~2 slots after its matmul."""
                obs = [out_pool.tile([128, 1024], BF16, tag="ob",
                                     name=f"ob{b}_{qb}_{ti}")
                       for ti in range(4)]
                pos = {}

                def op_mm(ti, j):
                    po = ps_po.tile([128, 512], F32, tag="po",
                                    name=f"po{b}_{qb}_{ti}_{j}")
                    nc.tensor.matmul(po[:],
                                     attn[:, 128 * ti:128 * ti + 128],
                                     wo[:, 512 * j:512 * j + 512],
                                     start=True, stop=True)
                    pos[(ti, j)] = po

                def op_cp(ti, j, on_act):
                    po = pos.pop((ti, j))
                    ob = obs[ti]
                    if on_act:
                        nc.scalar.activation(ob[:, 512 * j:512 * j + 512],
                                             po[:], CopyF)
                    else:
                        nc.vector.tensor_copy(ob[:, 512 * j:512 * j + 512],
                                              po[:])
                    if j == 1:
                        row0 = b * S + 512 * qb + 128 * ti
                        nc.sync.dma_start(out16_p[row0:row0 + 128, :], ob[:])

                halves = [(ti, j) for ti in range(4) for j in range(2)]
                mms = [lambda ti=ti, j=j: op_mm(ti, j) for ti, j in halves]
                cps = [lambda ti=ti, j=j, on_act=((ti + j) % 2 == 0):
                       op_cp(ti, j, on_act) for ti, j in halves]
                return mms, cps

            TRAIL = 2

            class Unit:
                """Per-unit attention state for the continuous chunk stream."""

                def __init__(self, b, qb):
                    self.b, self.qb = b, qb
                    self.av = None
                    self.ecs = {}

                def emit_scores_exp(self, c):
                    b, qb = self.b, self.qb
                    qsl = qt[b][qb]
                    tb_of_c = c // 2        # source projection block
                    cc = c % 2              # k-tile pair within block
                    for h in range(2):
                        ec = exp_pool.tile([128, 1024], F8, tag=f"expt{h}",
                                           name=f"ec{b}_{qb}_{c}_{h}")
                        if SPLIT_EXP:
                            for j in range(2):
                                sp = ps_sp.tile([128, 512], F32, tag="sp",
                                                bufs=SP_BUFS,
                                                name=f"sp{b}_{qb}_{c}_{h}_{j}")
                                kt_loc = 2 * cc + j
                                nc.tensor.matmul(
                                    sp[:],
                                    kt[b][tb_of_c][64 * h:64 * h + 64,
                                                   128 * kt_loc:128 * kt_loc + 128],
                                    qsl[64 * h:64 * h + 64, :],
                                    start=True, stop=True)
                                ech = ec[:, 512 * j:512 * j + 512]
                                if (c, h) in DVE_EXP:
                                    nc.vector._custom_dve(exp_op, out=ech,
                                                          in0=sp[:],
                                                          s0=exp_s0, s1=exp_s1)
                                else:
                                    nc.scalar.activation(
                                        ech, sp[:],
                                        mybir.ActivationFunctionType.Exp,
                                        scale=EXP_SCALE)
                        else:
                            sp = ps_sp.tile([128, 1024], F32, tag="sp", bufs=2,
                                            name=f"sp{b}_{qb}_{c}_{h}")
                            for j in range(2):
                                kt_loc = 2 * cc + j
                                nc.tensor.matmul(
                                    sp[:, 512 * j:512 * j + 512],
                                    kt[b][tb_of_c][64 * h:64 * h + 64,
                                                   128 * kt_loc:128 * kt_loc + 128],
                                    qsl[64 * h:64 * h + 64, :],
                                    start=True, stop=True)
                            if (c, h) in DVE_EXP:
                                nc.vector._custom_dve(exp_op, out=ec[:],
                                                      in0=sp[:],
                                                      s0=exp_s0, s1=exp_s1)
                            else:
                                nc.scalar.activation(
                                    ec[:], sp[:],
                                    mybir.ActivationFunctionType.Exp,
                                    scale=EXP_SCALE)
                        self.ecs[(c, h)] = ec

                def emit_av(self, cp):
                    b, qb = self.b, self.qb
                    if self.av is None:
                        self.av = [ps_av.tile([65, 512], F32, tag="av",
                                              name=f"av{b}_{qb}_{h}")
                                   for h in range(2)]
                    tb_p = cp // 2
                    ccp = cp % 2
                    va3 = vaug[b][tb_p].rearrange("p (k c) -> p k c", c=VG)
                    for h in range(2):
                        ec3 = self.ecs.pop((cp, h)).rearrange(
                            "p (k q) -> p k q", q=512)
                        nc.tensor.matmul(
                            self.av[h][:],
                            va3[:, 2 * ccp:2 * ccp + 2, HOFF * h:HOFF * h + 65],
                            ec3[:, :, :],
                            start=(cp == 0), stop=(cp == NCH - 1),
                            perf_mode=mybir.MatmulPerfMode.DoubleRow)

                def emit_normalize(self):
                    b, qb = self.b, self.qb
                    attn = attn_pool.tile([128, 512], BF16, tag="attn",
                                          name=f"attn{b}_{qb}")
                    rrows, bcs = [], []
                    for h in range(2):
                        rrow = small.tile([1, 512], F32, tag=f"rrow{h}",
                                          name=f"rr{b}_{qb}_{h}")
                        nc.vector.reciprocal(rrow[:], self.av[h][64:65, :])
                        rrows.append(rrow)
                    for h in range(2):
                        bc_sb = small.tile([64, 512], F32, tag=f"bc_sb{h}",
                                           name=f"bs{b}_{qb}_{h}")
                        nc.gpsimd.partition_broadcast(bc_sb[:], rrows[h])
                        bcs.append(bc_sb)
                    for h in range(2):
                        nc.vector.tensor_mul(attn[64 * h:64 * h + 64, :],
                                             self.av[h][0:64, :], bcs[h])
                    return attn

            # Continuous chunk stream: unit u's chunks 0..7 emit scores+exp;
            # its AV matmuls trail by TRAIL chunks, spilling into unit u+1's
            # first TRAIL chunks; normalize for unit u is emitted right after
            # its last AV (chunk TRAIL of unit u+1), and its out-proj drains
            # later in that unit via the bg queue. Projection blocks for
            # batch b+1 interleave throughout. No engine ever runs dry at a
            # unit boundary.
            from collections import deque
            bgA = deque()        # projection closures (internally ordered)
            bgM = deque()        # out-proj matmuls (producers)
            bgC = deque()        # out-proj PSUM->SBUF copies (consumers)
            nM = nC = 0          # popped counts for producer/consumer pacing
            units = [(b, qb) for b in range(B) for qb in range(QB_PER_B)]

            def drain_mc():
                nonlocal nM, nC
                # po pool is single-buffered: the copy of matmul i must be
                # emitted before matmul i+1 (WAR on the recycled bank)
                if bgC and (nM - nC >= 1 or not bgM):
                    bgC.popleft()()
                    nC += 1
                elif bgM:
                    bgM.popleft()()
                    nM += 1

            for tb in range(TB_PER_B):
                emit_proj_dma(0, tb)
            emit_proj_compute(0, 0)
            emit_proj_compute(0, 1)
            bgA.extend(proj_closures(0, 2))
            bgA.extend(proj_closures(0, 3))

            prev = None          # unit with AV trail / normalize outstanding
            pend_out = None      # out-proj closures awaiting queue insertion
            for u, (b, qb) in enumerate(units):
                cur = Unit(b, qb)
                if b + 1 < B:
                    emit_proj_dma(b + 1, qb)
                    bgA.extend(proj_closures(b + 1, qb))
                if pend_out is not None:
                    bgM.extend(pend_out[0])
                    bgC.extend(pend_out[1])
                    pend_out = None
                for c in range(NCH):
                    cur.emit_scores_exp(c)
                    if c >= TRAIL:
                        cur.emit_av(c - TRAIL)
                    elif prev is not None:
                        prev.emit_av(NCH - TRAIL + c)
                        if c == TRAIL - 1:
                            attn = prev.emit_normalize()
                            pend_out = outproj_closures(prev.b, prev.qb, attn)
                    # pace the projection queue so it fully drains within
                    # this unit: chunk c of NCH has (NCH - c) chunks left,
                    # and the next unit's trailing AVs / scores depend on
                    # this unit's projection outputs (kt, vaug)
                    na = max(2, -(-len(bgA) // (NCH - c)))
                    for _ in range(na):
                        if bgA:
                            bgA.popleft()()
                    drain_mc()
                    drain_mc()
                prev = cur
            for c in range(TRAIL):
                prev.emit_av(NCH - TRAIL + c)
            attn = prev.emit_normalize()
            if pend_out is not None:
                bgM.extend(pend_out[0])
                bgC.extend(pend_out[1])
            mo, co = outproj_closures(prev.b, prev.qb, attn)
            bgM.extend(mo)
            bgC.extend(co)
            while bgA or bgM or bgC:
                if bgA:
                    bgA.popleft()()
                drain_mc()

    nc.compile()
    return nc


def _get_compiled():
    global _COMPILED
    if _COMPILED is None:
        _COMPILED = _build()
    return _COMPILED


def _prep_inputs(x, Wq, bq, Wk, bk, Wv, bv, Wo, bo):
    xf = np.asarray(x, dtype=np.float32).reshape(NT, D).T
    xt = np.ascontiguousarray(xf).astype(bf16)

    def pack_w(Wc, dtype, scale=1.0):
        # [128 out, 1024 in] -> k-tile packed [128, 1024]
        wt = np.asarray(Wc, dtype=np.float32).T * scale  # [1024 in, 128 out]
        return np.ascontiguousarray(
            wt.reshape(8, 128, 128).transpose(1, 0, 2).reshape(128, D)).astype(dtype)

    if QK_DR:
        xt8 = np.ascontiguousarray(xf).astype(f8)

    in_maps = []
    for c in range(N_CORES):
        sl = slice(128 * c, 128 * c + 128)
        m = {
            "xt": xt,
            "wv": pack_w(np.asarray(Wv)[sl], bf16),
            "wo": np.ascontiguousarray(
                np.asarray(Wo, dtype=np.float32)[:, sl].T).astype(bf16),
            "bv": np.asarray(bv, dtype=np.float32)[sl].reshape(128, 1),
        }
        if QK_DR:
            m["xt8"] = xt8
            m["wq"] = pack_w(np.asarray(Wq)[sl], f8, QK_W_SCALE)
            m["wk"] = pack_w(np.asarray(Wk)[sl], f8, QK_W_SCALE)
            m["bq"] = (np.asarray(bq, dtype=np.float32)[sl] *
                       QK_W_SCALE).reshape(128, 1)
            m["bk"] = (np.asarray(bk, dtype=np.float32)[sl] *
                       QK_W_SCALE).reshape(128, 1)
        else:
            m["wq"] = pack_w(np.asarray(Wq)[sl], bf16)
            m["wk"] = pack_w(np.asarray(Wk)[sl], bf16)
            m["bq"] = np.asarray(bq, dtype=np.float32)[sl].reshape(128, 1)
            m["bk"] = np.asarray(bk, dtype=np.float32)[sl].reshape(128, 1)
        in_maps.append(m)
    return in_maps


def kernel(x, Wq, bq, Wk, bk, Wv, bv, Wo, bo):
    nc = _get_compiled()
    in_maps = _prep_inputs(x, Wq, bq, Wk, bk, Wv, bv, Wo, bo)
    res = run_bass_kernel_spmd(nc, in_maps, core_ids=list(range(N_CORES)))
    acc = np.zeros((NT, D), dtype=np.float32)
    for c in range(N_CORES):
        acc += np.asarray(res.results[c]["out16"]).astype(np.float32)
    acc += np.asarray(bo, dtype=np.float32)[None, :]
    return acc.reshape(B, S, D)


# revision 31
# speedup vs baseline: 1.0284x; 1.0284x over previous
"""MultiHeadAttention Trainium2 kernel, 8-way tensor-parallel by head.

Problem: B=4, S=2048, D=1024, 16 heads, d_k=64 (nn_MultiHeadAttention_67585605370071).

Sharding: each core owns 2 heads (128 of the 1024 hidden dims):
  Wq/Wk/Wv column-sharded by head, Wo row-sharded; the 8 partial outputs
  are summed on the host (the row-shard reduction) and bo is added there.

Dataflow per core (matmuls bf16 / fp8-DoubleRow, f32 PSUM accumulation):
  - host passes x pre-transposed (xT bf16 [1024, 8192]); Q/K/V projections
    run bf16 (QK_DR=True switches Q/K to fp8 DoubleRow for ~4us, at the
    cost of rel-err 1.5e-2 -> 2.0e-2)
  - V rotated to token-major layout via SBUF->SBUF DMA transposes, then
    quantized fp8 into the augmented layout [V_A | 1 | pad | V_B | 1 | pad]
    by the GpSimd engine; the ones column makes the AV matmul emit softmax
    denominators as row 64 of its PSUM tile
  - scoresT = Kt.T @ Qt per [128,512] half-tile; exp is split between the
    ScalarE ACT (table exp) and a custom DVE microcode op EXP16_POLY_ANT
    ((1 + ks + (ks)^2/2)^16, 8/8 uop stages) so neither engine saturates;
    both emit fp8
  - AV via fp8 DoubleRow matmuls (2 k-tiles per pass), trailing the
    scores/exp stream by TRAIL chunks so the PE never stalls on exp latency
  - normalize via DVE reciprocal + GpSimd partition_broadcast + DVE
    tensor_mul (no PE broadcast matmul)
  - out-proj per 128-token tile; PSUM->SBUF copies alternate between an ACT
    Copy and a DVE tensor_copy; projection bias-adds run as ACT
    Identity+bias (exp/identity/copy share one ACT table - no reloads)
  - emission is a continuous chunk stream: unit u's trailing AVs and
    normalize ride in unit u+1's first chunks, projection closures drain
    with ceiling pacing (fully inside their unit - the next unit's scores
    and trailing AVs depend on kt/vaug), and out-proj matmul/copy pairs
    drain through paced producer/consumer queues that keep the single
    po PSUM bank's write-after-read order identical to emission order
PSUM budget (8 banks): 4x score [128,512] + 2x AV accumulators [65,512]
  + 1 projection bank + 1 out-proj bank.
"""
import numpy as np
import ml_dtypes

import concourse.bass as bass
import concourse.bacc as bacc
import concourse.mybir as mybir
import concourse.tile as tile
from concourse.bass_utils import run_bass_kernel_spmd

import concourse.dve_ops as dve_ops_mod
from concourse.dve_spec import C0, C1, One, Spec, Src0, lower as dve_lower, sq
from concourse.dve_uop import DveOpSpec

BF16 = mybir.dt.bfloat16
F32 = mybir.dt.float32
F8 = mybir.dt.float8e4
bf16 = ml_dtypes.bfloat16
f8 = ml_dtypes.float8_e4m3

B, S, D = 4, 2048, 1024
NT = B * S            # 8192 tokens
N_CORES = 8
KT_PER_B = S // 128   # 16 k-tiles per batch
QB_PER_B = S // 512   # 4 query blocks per batch
TB_PER_B = S // 512   # 4 token blocks per batch (projection)
NCH = KT_PER_B // 2   # 8 score/exp chunks per attention unit

# fp8-e4m3 attention-weights path: expT and V quantized to fp8 so the AV
# matmul can use DoubleRow (2x PE throughput).
VG = 144              # vaug per-k-tile column group
HOFF = 72             # head B column offset inside a group

# Q/K projections in fp8 DoubleRow; weights scaled by 32 on the host, the
# 32*32 factor folded into the exp scale.
QK_DR = False
QK_W_SCALE = 32.0
EXP_SCALE = 0.125 / (QK_W_SCALE * QK_W_SCALE if QK_DR else 1.0)

# exp tiles handed to the DVE custom op instead of ScalarE, per unit
DVE_EXP = {(c, 1) for c in range(7)}
SPLIT_EXP = True     # exp per [128,512] half (4 sp bufs) vs full [128,1024]
SPLIT_OUTPROJ = False  # out-proj mm/copy as separate bg closures
NDRAIN = 3           # bg closures drained per chunk
SP_BUFS = 4          # score-PSUM buffers ([128,512] each)
TRAIL_G = 4          # chunks the AV stream trails scores/exp by
MC_POPS = 2          # out-proj producer/consumer pops per chunk
SH_BUFS = 2          # shared pp/po PSUM buffers

_COMPILED = None

# ---------------------------------------------------------------------------
# Custom DVE exp op: out = exp(in * scale) via (1 + ks + (ks)^2/2)^16,
# k = scale/16 — quadratic (4 ALU stages) + 4 squarings = 8/8 v3 stages.
# Rel err vs true exp: rms ~1e-4; after fp8 output quantization 99.96% of
# outputs are bit-identical to fp8(exp(x)).
EXP16_NAME = "EXP16_POLY_ANT"


def _exp16_consts(scale):
    k = scale / 16.0
    return (k * k / 2.0, k)


def _exp16_ref(in0, in1, s0, s1, imm2):
    b = ((in0.astype(np.float32) * s0 + s1) * in0 + 1.0).astype(np.float32)
    for _ in range(4):
        b = (b * b).astype(np.float32)
    return b


def _register_exp16():
    for op in dve_ops_mod.OPS:
        if op.name == EXP16_NAME:
            return op
    body = sq(sq(sq(sq((Src0 * C0 + C1) * Src0 + One))))
    spec = Spec(body=body, reference=_exp16_ref)
    row = max(dve_ops_mod._SUB_OPCODE_FOR_NAME.values()) + 1
    assert row < 0x20, "custom-DVE opcode rows exhausted"
    dve_ops_mod._SUB_OPCODE_FOR_NAME[EXP16_NAME] = row
    shas = {}
    for ver in ("v3", "v4"):
        s = DveOpSpec(name=EXP16_NAME, opcode=row,
                      uops=dve_lower(spec, ver=ver), rd1_en=False)
        shas[ver] = s.sha(ver)
    op = dve_ops_mod.DveOp(EXP16_NAME, spec, False, shas)
    dve_ops_mod.OPS.append(op)
    dve_ops_mod.CUSTOM_DVE_SPECS[EXP16_NAME] = spec
    return op


# ---------------------------------------------------------------------------
def _build():
    exp_op = _register_exp16()
    exp_s0, exp_s1 = _exp16_consts(EXP_SCALE)
    Ident = mybir.ActivationFunctionType.Identity
    CopyF = mybir.ActivationFunctionType.Copy

    nc = bacc.Bacc("TRN2", target_bir_lowering=False, debug=False,
                   num_devices=N_CORES)

    xt_p = nc.declare_dram_parameter("xt", [D, NT], BF16, isOutput=False)
    wv_p = nc.declare_dram_parameter("wv", [128, D], BF16, isOutput=False)
    wo_p = nc.declare_dram_parameter("wo", [128, D], BF16, isOutput=False)
    bq_p = nc.declare_dram_parameter("bq", [128, 1], F32, isOutput=False)
    bk_p = nc.declare_dram_parameter("bk", [128, 1], F32, isOutput=False)
    bv_p = nc.declare_dram_parameter("bv", [128, 1], F32, isOutput=False)
    if QK_DR:
        xt8_p = nc.declare_dram_parameter("xt8", [D, NT], F8, isOutput=False)
        wq_p = nc.declare_dram_parameter("wq", [128, D], F8, isOutput=False)
        wk_p = nc.declare_dram_parameter("wk", [128, D], F8, isOutput=False)
    else:
        wq_p = nc.declare_dram_parameter("wq", [128, D], BF16, isOutput=False)
        wk_p = nc.declare_dram_parameter("wk", [128, D], BF16, isOutput=False)
    out16_p = nc.declare_dram_parameter("out16", [NT, D], BF16, isOutput=True)

    WDT = F8 if QK_DR else BF16

    with tile.TileContext(nc) as tc:
        with (
            tc.tile_pool(name="consts", bufs=1) as consts,
            tc.tile_pool(name="xts", bufs=3) as xts_pool,
            tc.tile_pool(name="qkv", bufs=2) as qkv_pool,
            tc.tile_pool(name="vtrp", bufs=4) as vtr_pool,
            tc.tile_pool(name="expp", bufs=4) as exp_pool,
            tc.tile_pool(name="attnp", bufs=3) as attn_pool,
            tc.tile_pool(name="small", bufs=3) as small,
            tc.tile_pool(name="outp", bufs=3) as out_pool,
            tc.tile_pool(name="ps_sp", bufs=2, space=bass.MemorySpace.PSUM) as ps_sp,
            tc.tile_pool(name="ps_av", bufs=2, space=bass.MemorySpace.PSUM) as ps_av,
            tc.tile_pool(name="ps_pp", bufs=1, space=bass.MemorySpace.PSUM) as ps_pp,
            tc.tile_pool(name="ps_po", bufs=1, space=bass.MemorySpace.PSUM) as ps_po,
        ):
            wq = consts.tile([128, D], WDT, name="wq")
            wk = consts.tile([128, D], WDT, name="wk")
            wv = consts.tile([128, D], BF16, name="wv")
            wo = consts.tile([128, D], BF16, name="wo")
            bq = consts.tile([128, 1], F32, name="bq")
            bk = consts.tile([128, 1], F32, name="bk")
            bv = consts.tile([128, 1], F32, name="bv")
            nc.sync.dma_start(wq[:], wq_p[:])
            nc.sync.dma_start(wk[:], wk_p[:])
            nc.sync.dma_start(wv[:], wv_p[:])
            nc.sync.dma_start(wo[:], wo_p[:])
            nc.sync.dma_start(bq[:], bq_p[:])
            nc.sync.dma_start(bk[:], bk_p[:])
            nc.sync.dma_start(bv[:], bv_p[:])

            # per-batch, per-block persistent tiles, allocated lazily
            qt = {}          # qt[b][tb] -> [128, 512]
            kt = {}          # kt[b][tb] -> [128, 512] (4 k-tiles each)
            vts = {}         # vts[b][tb] -> [128, 512]
            vaug = {}        # vaug[b][tb] -> [128, 4*VG] fp8
            xts_t = {}       # prefetched xT stacks (bf16, fp8)

            def _alloc_batch(b):
                qt[b] = [qkv_pool.tile([128, 512], BF16, tag=f"qt{t}",
                                       name=f"qt{b}_{t}") for t in range(4)]
                kt[b] = [qkv_pool.tile([128, 512], BF16, tag=f"kt{t}",
                                       name=f"kt{b}_{t}") for t in range(4)]
                vts[b] = [qkv_pool.tile([128, 512], BF16, tag=f"vts{t}",
                                        name=f"vts{b}_{t}") for t in range(4)]
                vaug[b] = []
                for t in range(4):
                    va = qkv_pool.tile([128, 4 * VG], F8, tag=f"vaug{t}",
                                       name=f"vaug{b}_{t}")
                    va3 = va.rearrange("p (k c) -> p k c", c=VG)
                    nc.gpsimd.memset(va3[:, :, 64:65], 1.0)
                    nc.gpsimd.memset(va3[:, :, HOFF + 64:HOFF + 65], 1.0)
                    vaug[b].append(va)

            def emit_proj_dma(b, tb):
                """Prefetch the 512-token xT stack(s) for block (b, tb)."""
                if b not in qt:
                    _alloc_batch(b)
                tok0 = b * S + tb * 512
                xts = xts_pool.tile([128, 8 * 512], BF16, tag="xts",
                                    name=f"xts{b}_{tb}")
                src3 = xt_p.rearrange("(kd p) t -> p kd t", p=128)
                dst3 = xts.rearrange("p (kd t) -> p kd t", t=512)
                nc.sync.dma_start(dst3[:, :, :], src3[:, :, tok0:tok0 + 512])
                if QK_DR:
                    xts8 = xts_pool.tile([128, 8 * 512], F8, tag="xts8",
                                         name=f"xts8{b}_{tb}")
                    src83 = xt8_p.rearrange("(kd p) t -> p kd t", p=128)
                    dst83 = xts8.rearrange("p (kd t) -> p kd t", t=512)
                    nc.sync.dma_start(dst83[:, :, :], src83[:, :, tok0:tok0 + 512])
                    xts_t[(b, tb)] = (xts, xts8)
                else:
                    xts_t[(b, tb)] = (xts, xts)

            def emit_proj_compute(b, tb):
                for clo in proj_closures(b, tb):
                    clo()

            def proj_closures(b, tb):
                """The projection block as a list of small closures so its PE
                work can be interleaved between attention score chunks.
                V runs first so its rotation DMAs are in flight long before
                the Pool quantize copies (emitted last) need them."""
                xts, xts8 = xts_t.pop((b, tb))

                # Build producer (PE matmul) and consumer (ACT bias / Pool
                # quantize) closures, then interleave so every consumer
                # drains a few slots after its producer — an in-order
                # engine never head-of-line blocks on a PE matmul.
                ppk = ps_pp.tile([128, 512], F32, tag="pp", name=f"ppk{b}_{tb}")
                ppq = ps_pp.tile([128, 512], F32, tag="pp", name=f"ppq{b}_{tb}")
                ppv = ps_pp.tile([128, 512], F32, tag="pp", name=f"ppv{b}_{tb}")

                def mk_qk(pp, w_sb):
                    if QK_DR:
                        w3 = w_sb.rearrange("p (k m) -> p k m", m=128)
                        x3 = xts8.rearrange("p (k t) -> p k t", t=512)

                        def mmdr(p0):
                            for p in (p0, p0 + 1):
                                nc.tensor.matmul(
                                    pp[:], w3[:, 2 * p:2 * p + 2, :],
                                    x3[:, 2 * p:2 * p + 2, :],
                                    start=(p == 0), stop=(p == 3),
                                    perf_mode=mybir.MatmulPerfMode.DoubleRow)
                        return [lambda p0=p0: mmdr(p0) for p0 in (0, 2)]

                    def mm2(kd0):
                        for kd in (kd0, kd0 + 1):
                            nc.tensor.matmul(
                                pp[:], w_sb[:, 128 * kd:128 * kd + 128],
                                xts[:, 512 * kd:512 * kd + 512],
                                start=(kd == 0), stop=(kd == 7))
                    return [lambda kd0=kd0: mm2(kd0) for kd0 in (0, 2, 4, 6)]

                def mm2v(kd0):
                    for kd in (kd0, kd0 + 1):
                        nc.tensor.matmul(
                            ppv[:], wv[:, 128 * kd:128 * kd + 128],
                            xts[:, 512 * kd:512 * kd + 512],
                            start=(kd == 0), stop=(kd == 7))

                vtrs = [vtr_pool.tile([128, 128], BF16, tag=f"vtr{ti}",
                                      name=f"vtr{b}_{tb}_{ti}")
                        for ti in range(4)]

                def rot_dma():
                    for ti in range(4):
                        nc.sync.dma_start_transpose(
                            vtrs[ti][:], vts[b][tb][:, 128 * ti:128 * ti + 128])

                va4 = vaug[b][tb].rearrange("p (k g e) -> p k g e", g=2, e=HOFF)

                def rot_q(ti):
                    vtr3 = vtrs[ti].rearrange("p (g e) -> p g e", e=64)
                    nc.gpsimd.tensor_copy(va4[:, ti, :, 0:64], vtr3[:, :, :])

                kbias = lambda: nc.scalar.activation(kt[b][tb][:], ppk[:],
                                                     Ident, bias=bk[:])
                qbias = lambda: nc.scalar.activation(qt[b][tb][:], ppq[:],
                                                     Ident, bias=bq[:])
                vbias = lambda: nc.scalar.activation(vts[b][tb][:], ppv[:],
                                                     Ident, bias=bv[:])
                kmm = mk_qk(ppk, wk)
                qmm = mk_qk(ppq, wq)
                vmm = [lambda kd0=kd0: mm2v(kd0) for kd0 in (0, 2, 4, 6)]
                return (kmm + [kbias] + qmm + [qbias] + vmm
                        + [vbias, rot_dma]
                        + [lambda ti=ti: rot_q(ti) for ti in range(4)])

            def outproj_closures(b, qb, attn):
                """Interleaved [mm, mm, copy, mm, copy, ...] so each
                PSUM->SBUF copy drains ~2 slots after its matmul."""
                obs = [out_pool.tile([128, 1024], BF16, tag="ob",
                                     name=f"ob{b}_{qb}_{ti}")
                       for ti in range(4)]
                pos = {}

                def op_mm(ti, j):
                    po = ps_po.tile([128, 512], F32, tag="po",
                                    name=f"po{b}_{qb}_{ti}_{j}")
                    nc.tensor.matmul(po[:],
                                     attn[:, 128 * ti:128 * ti + 128],
                                     wo[:, 512 * j:512 * j + 512],
                                     start=True, stop=True)
                    pos[(ti, j)] = po

                def op_cp(ti, j, on_act):
                    po = pos.pop((ti, j))
                    ob = obs[ti]
                    if on_act:
                        nc.scalar.activation(ob[:, 512 * j:512 * j + 512],
                                             po[:], CopyF)
                    else:
                        nc.vector.tensor_copy(ob[:, 512 * j:512 * j + 512],
                                              po[:])
                    if j == 1:
                        row0 = b * S + 512 * qb + 128 * ti
                        nc.sync.dma_start(out16_p[row0:row0 + 128, :], ob[:])

                halves = [(ti, j) for ti in range(4) for j in range(2)]
                mms = [lambda ti=ti, j=j: op_mm(ti, j) for ti, j in halves]
                cps = [lambda ti=ti, j=j, on_act=((ti + j) % 2 == 0):
                       op_cp(ti, j, on_act) for ti, j in halves]
                return mms, cps

            TRAIL = TRAIL_G

            class Unit:
                """Per-unit attention state for the continuous chunk stream."""

                def __init__(self, b, qb):
                    self.b, self.qb = b, qb
                    self.av = None
                    self.ecs = {}

                def emit_scores_exp(self, c):
                    b, qb = self.b, self.qb
                    qsl = qt[b][qb]
                    tb_of_c = c // 2        # source projection block
                    cc = c % 2              # k-tile pair within block
                    for h in range(2):
                        ec = exp_pool.tile([128, 1024], F8, tag=f"expt{h}",
                                           bufs=TRAIL_G + 2,
                                           name=f"ec{b}_{qb}_{c}_{h}")
                        if SPLIT_EXP:
                            for j in range(2):
                                sp = ps_sp.tile([128, 512], F32, tag="sp",
                                                bufs=SP_BUFS,
                                                name=f"sp{b}_{qb}_{c}_{h}_{j}")
                                kt_loc = 2 * cc + j
                                nc.tensor.matmul(
                                    sp[:],
                                    kt[b][tb_of_c][64 * h:64 * h + 64,
                                                   128 * kt_loc:128 * kt_loc + 128],
                                    qsl[64 * h:64 * h + 64, :],
                                    start=True, stop=True)
                                ech = ec[:, 512 * j:512 * j + 512]
                                if (c, h) in DVE_EXP:
                                    nc.vector._custom_dve(exp_op, out=ech,
                                                          in0=sp[:],
                                                          s0=exp_s0, s1=exp_s1)
                                else:
                                    nc.scalar.activation(
                                        ech, sp[:],
                                        mybir.ActivationFunctionType.Exp,
                                        scale=EXP_SCALE)
                        else:
                            sp = ps_sp.tile([128, 1024], F32, tag="sp", bufs=2,
                                            name=f"sp{b}_{qb}_{c}_{h}")
                            for j in range(2):
                                kt_loc = 2 * cc + j
                                nc.tensor.matmul(
                                    sp[:, 512 * j:512 * j + 512],
                                    kt[b][tb_of_c][64 * h:64 * h + 64,
                                                   128 * kt_loc:128 * kt_loc + 128],
                                    qsl[64 * h:64 * h + 64, :],
                                    start=True, stop=True)
                            if (c, h) in DVE_EXP:
                                nc.vector._custom_dve(exp_op, out=ec[:],
                                                      in0=sp[:],
                                                      s0=exp_s0, s1=exp_s1)
                            else:
                                nc.scalar.activation(
                                    ec[:], sp[:],
                                    mybir.ActivationFunctionType.Exp,
                                    scale=EXP_SCALE)
                        self.ecs[(c, h)] = ec

                def emit_av(self, cp):
                    b, qb = self.b, self.qb
                    if self.av is None:
                        self.av = [ps_av.tile([65, 512], F32, tag="av",
                                              name=f"av{b}_{qb}_{h}")
                                   for h in range(2)]
                    tb_p = cp // 2
                    ccp = cp % 2
                    va3 = vaug[b][tb_p].rearrange("p (k c) -> p k c", c=VG)
                    for h in range(2):
                        ec3 = self.ecs.pop((cp, h)).rearrange(
                            "p (k q) -> p k q", q=512)
                        nc.tensor.matmul(
                            self.av[h][:],
                            va3[:, 2 * ccp:2 * ccp + 2, HOFF * h:HOFF * h + 65],
                            ec3[:, :, :],
                            start=(cp == 0), stop=(cp == NCH - 1),
                            perf_mode=mybir.MatmulPerfMode.DoubleRow)

                def emit_normalize(self):
                    b, qb = self.b, self.qb
                    attn = attn_pool.tile([128, 512], BF16, tag="attn",
                                          name=f"attn{b}_{qb}")
                    rrows, bcs = [], []
                    for h in range(2):
                        rrow = small.tile([1, 512], F32, tag=f"rrow{h}",
                                          name=f"rr{b}_{qb}_{h}")
                        nc.vector.reciprocal(rrow[:], self.av[h][64:65, :])
                        rrows.append(rrow)
                    for h in range(2):
                        bc_sb = small.tile([64, 512], F32, tag=f"bc_sb{h}",
                                           name=f"bs{b}_{qb}_{h}")
                        nc.gpsimd.partition_broadcast(bc_sb[:], rrows[h])
                        bcs.append(bc_sb)
                    for h in range(2):
                        nc.vector.tensor_mul(attn[64 * h:64 * h + 64, :],
                                             self.av[h][0:64, :], bcs[h])
                    return attn

            # Continuous chunk stream: unit u's chunks 0..7 emit scores+exp;
            # its AV matmuls trail by TRAIL chunks, spilling into unit u+1's
            # first TRAIL chunks; normalize for unit u is emitted right after
            # its last AV (chunk TRAIL of unit u+1), and its out-proj drains
            # later in that unit via the bg queue. Projection blocks for
            # batch b+1 interleave throughout. No engine ever runs dry at a
            # unit boundary.
            from collections import deque
            bgA = deque()        # projection closures (internally ordered)
            bgM = deque()        # out-proj matmuls (producers)
            bgC = deque()        # out-proj PSUM->SBUF copies (consumers)
            nM = nC = 0          # popped counts for producer/consumer pacing
            units = [(b, qb) for b in range(B) for qb in range(QB_PER_B)]

            def drain_mc():
                nonlocal nM, nC
                # po pool is single-buffered: the copy of matmul i must be
                # emitted before matmul i+1 (WAR on the recycled bank)
                if bgC and (nM - nC >= 1 or not bgM):
                    bgC.popleft()()
                    nC += 1
                elif bgM:
                    bgM.popleft()()
                    nM += 1

            for tb in range(TB_PER_B):
                emit_proj_dma(0, tb)
            emit_proj_compute(0, 0)
            emit_proj_compute(0, 1)
            bgA.extend(proj_closures(0, 2))
            bgA.extend(proj_closures(0, 3))

            prev = None          # unit with AV trail / normalize outstanding
            pend_out = None      # out-proj closures awaiting queue insertion
            for u, (b, qb) in enumerate(units):
                cur = Unit(b, qb)
                if b + 1 < B:
                    emit_proj_dma(b + 1, qb)
                    bgA.extend(proj_closures(b + 1, qb))
                if pend_out is not None:
                    bgM.extend(pend_out[0])
                    bgC.extend(pend_out[1])
                    pend_out = None
                for c in range(NCH):
                    cur.emit_scores_exp(c)
                    if c >= TRAIL:
                        cur.emit_av(c - TRAIL)
                    elif prev is not None:
                        prev.emit_av(NCH - TRAIL + c)
                        if c == TRAIL - 1:
                            attn = prev.emit_normalize()
                            pend_out = outproj_closures(prev.b, prev.qb, attn)
                    # pace the projection queue so it fully drains within
                    # this unit: chunk c of NCH has (NCH - c) chunks left,
                    # and the next unit's trailing AVs / scores depend on
                    # this unit's projection outputs (kt, vaug)
                    na = max(2, -(-len(bgA) // (NCH - c)))
                    for _ in range(na):
                        if bgA:
                            bgA.popleft()()
                    for _ in range(MC_POPS):
                        drain_mc()
                prev = cur
            for c in range(TRAIL):
                prev.emit_av(NCH - TRAIL + c)
            attn = prev.emit_normalize()
            if pend_out is not None:
                bgM.extend(pend_out[0])
                bgC.extend(pend_out[1])
            mo, co = outproj_closures(prev.b, prev.qb, attn)
            bgM.extend(mo)
            bgC.extend(co)
            while bgA or bgM or bgC:
                if bgA:
                    bgA.popleft()()
                drain_mc()

    nc.compile()
    return nc


def _get_compiled():
    global _COMPILED
    if _COMPILED is None:
        _COMPILED = _build()
    return _COMPILED


def _prep_inputs(x, Wq, bq, Wk, bk, Wv, bv, Wo, bo):
    xf = np.asarray(x, dtype=np.float32).reshape(NT, D).T
    xt = np.ascontiguousarray(xf).astype(bf16)

    def pack_w(Wc, dtype, scale=1.0):
        # [128 out, 1024 in] -> k-tile packed [128, 1024]
        wt = np.asarray(Wc, dtype=np.float32).T * scale  # [1024 in, 128 out]
        return np.ascontiguousarray(
            wt.reshape(8, 128, 128).transpose(1, 0, 2).reshape(128, D)).astype(dtype)

    if QK_DR:
        xt8 = np.ascontiguousarray(xf).astype(f8)

    in_maps = []
    for c in range(N_CORES):
        sl = slice(128 * c, 128 * c + 128)
        m = {
            "xt": xt,
            "wv": pack_w(np.asarray(Wv)[sl], bf16),
            "wo": np.ascontiguousarray(
                np.asarray(Wo, dtype=np.float32)[:, sl].T).astype(bf16),
            "bv": np.asarray(bv, dtype=np.float32)[sl].reshape(128, 1),
        }
        if QK_DR:
            m["xt8"] = xt8
            m["wq"] = pack_w(np.asarray(Wq)[sl], f8, QK_W_SCALE)
            m["wk"] = pack_w(np.asarray(Wk)[sl], f8, QK_W_SCALE)
            m["bq"] = (np.asarray(bq, dtype=np.float32)[sl] *
                       QK_W_SCALE).reshape(128, 1)
            m["bk"] = (np.asarray(bk, dtype=np.float32)[sl] *
                       QK_W_SCALE).reshape(128, 1)
        else:
            m["wq"] = pack_w(np.asarray(Wq)[sl], bf16)
            m["wk"] = pack_w(np.asarray(Wk)[sl], bf16)
            m["bq"] = np.asarray(bq, dtype=np.float32)[sl].reshape(128, 1)
            m["bk"] = np.asarray(bk, dtype=np.float32)[sl].reshape(128, 1)
        in_maps.append(m)
    return in_maps


def kernel(x, Wq, bq, Wk, bk, Wv, bv, Wo, bo):
    nc = _get_compiled()
    in_maps = _prep_inputs(x, Wq, bq, Wk, bk, Wv, bv, Wo, bo)
    res = run_bass_kernel_spmd(nc, in_maps, core_ids=list(range(N_CORES)))
    acc = np.zeros((NT, D), dtype=np.float32)
    for c in range(N_CORES):
        acc += np.asarray(res.results[c]["out16"]).astype(np.float32)
    acc += np.asarray(bo, dtype=np.float32)[None, :]
    return acc.reshape(B, S, D)


# revision 36
# speedup vs baseline: 1.0382x; 1.0095x over previous
"""MultiHeadAttention Trainium2 kernel, 8-way tensor-parallel by head.

Problem: B=4, S=2048, D=1024, 16 heads, d_k=64 (nn_MultiHeadAttention_67585605370071).

Sharding: each core owns 2 heads (128 of the 1024 hidden dims):
  Wq/Wk/Wv column-sharded by head, Wo row-sharded; the 8 partial outputs
  are summed on the host (the row-shard reduction) and bo is added there.

Dataflow per core (matmuls bf16 / fp8-DoubleRow, f32 PSUM accumulation):
  - host passes x pre-transposed (xT bf16 [1024, 8192]); Q/K/V projections
    run bf16 (QK_DR=True switches Q/K to fp8 DoubleRow for ~4us, at the
    cost of rel-err 1.5e-2 -> 2.0e-2)
  - V rotated to token-major layout via SBUF->SBUF DMA transposes, then
    quantized fp8 into the augmented layout [V_A | 1 | pad | V_B | 1 | pad]
    by the GpSimd engine; the ones column makes the AV matmul emit softmax
    denominators as row 64 of its PSUM tile
  - scoresT = Kt.T @ Qt per [128,512] half-tile; exp is split between the
    ScalarE ACT (table exp) and a custom DVE microcode op EXP16_POLY_ANT
    ((1 + ks + (ks)^2/2)^16, 8/8 uop stages) so neither engine saturates;
    both emit fp8
  - AV via fp8 DoubleRow matmuls (2 k-tiles per pass), trailing the
    scores/exp stream by TRAIL chunks so the PE never stalls on exp latency
  - normalize via DVE reciprocal + GpSimd partition_broadcast + DVE
    tensor_mul (no PE broadcast matmul)
  - out-proj per 128-token tile; PSUM->SBUF copies alternate between an ACT
    Copy and a DVE tensor_copy; projection bias-adds run as ACT
    Identity+bias (exp/identity/copy share one ACT table - no reloads)
  - emission is a continuous chunk stream: unit u's trailing AVs and
    normalize ride in unit u+1's first chunks, projection closures drain
    with ceiling pacing (fully inside their unit - the next unit's scores
    and trailing AVs depend on kt/vaug), and out-proj matmul/copy pairs
    drain through paced producer/consumer queues that keep the single
    po PSUM bank's write-after-read order identical to emission order
PSUM budget (8 banks): 4x score [128,512] + 2x AV accumulators [65,512]
  + 1 projection bank + 1 out-proj bank.
"""
import numpy as np
import ml_dtypes

import concourse.bass as bass
import concourse.bacc as bacc
import concourse.mybir as mybir
import concourse.tile as tile
from concourse.bass_utils import run_bass_kernel_spmd

import concourse.dve_ops as dve_ops_mod
from concourse.dve_spec import C0, C1, One, Spec, Src0, lower as dve_lower, sq
from concourse.dve_uop import DveOpSpec

BF16 = mybir.dt.bfloat16
F32 = mybir.dt.float32
F8 = mybir.dt.float8e4
bf16 = ml_dtypes.bfloat16
f8 = ml_dtypes.float8_e4m3

B, S, D = 4, 2048, 1024
NT = B * S            # 8192 tokens
N_CORES = 8
KT_PER_B = S // 128   # 16 k-tiles per batch
QB_PER_B = S // 512   # 4 query blocks per batch
TB_PER_B = S // 512   # 4 token blocks per batch (projection)
NCH = KT_PER_B // 2   # 8 score/exp chunks per attention unit

# fp8-e4m3 attention-weights path: expT and V quantized to fp8 so the AV
# matmul can use DoubleRow (2x PE throughput).
VG = 144              # vaug per-k-tile column group
HOFF = 72             # head B column offset inside a group

# Q/K projections in fp8 DoubleRow; weights scaled by 32 on the host, the
# 32*32 factor folded into the exp scale.
QK_DR = False
QK_W_SCALE = 32.0
EXP_SCALE = 0.125 / (QK_W_SCALE * QK_W_SCALE if QK_DR else 1.0)

# exp tiles handed to the DVE custom op instead of ScalarE, per unit
DVE_EXP = {(c, 1) for c in range(7)}
SPLIT_EXP = True     # exp per [128,512] half (4 sp bufs) vs full [128,1024]
SPLIT_OUTPROJ = False  # out-proj mm/copy as separate bg closures
NDRAIN = 3           # bg closures drained per chunk
SP_BUFS = 4          # score-PSUM buffers ([128,512] each)
TRAIL_G = 5          # chunks the AV stream trails scores/exp by
MC_POPS = 2          # out-proj producer/consumer pops per chunk
SH_BUFS = 2          # shared pp/po PSUM buffers

_COMPILED = None

# ---------------------------------------------------------------------------
# Custom DVE exp op: out = exp(in * scale) via (1 + ks + (ks)^2/2)^16,
# k = scale/16 — quadratic (4 ALU stages) + 4 squarings = 8/8 v3 stages.
# Rel err vs true exp: rms ~1e-4; after fp8 output quantization 99.96% of
# outputs are bit-identical to fp8(exp(x)).
EXP16_NAME = "EXP16_POLY_ANT"


def _exp16_consts(scale):
    k = scale / 16.0
    return (k * k / 2.0, k)


def _exp16_ref(in0, in1, s0, s1, imm2):
    b = ((in0.astype(np.float32) * s0 + s1) * in0 + 1.0).astype(np.float32)
    for _ in range(4):
        b = (b * b).astype(np.float32)
    return b


def _register_exp16():
    for op in dve_ops_mod.OPS:
        if op.name == EXP16_NAME:
            return op
    body = sq(sq(sq(sq((Src0 * C0 + C1) * Src0 + One))))
    spec = Spec(body=body, reference=_exp16_ref)
    row = max(dve_ops_mod._SUB_OPCODE_FOR_NAME.values()) + 1
    assert row < 0x20, "custom-DVE opcode rows exhausted"
    dve_ops_mod._SUB_OPCODE_FOR_NAME[EXP16_NAME] = row
    shas = {}
    for ver in ("v3", "v4"):
        s = DveOpSpec(name=EXP16_NAME, opcode=row,
                      uops=dve_lower(spec, ver=ver), rd1_en=False)
        shas[ver] = s.sha(ver)
    op = dve_ops_mod.DveOp(EXP16_NAME, spec, False, shas)
    dve_ops_mod.OPS.append(op)
    dve_ops_mod.CUSTOM_DVE_SPECS[EXP16_NAME] = spec
    return op


# ---------------------------------------------------------------------------
def _build():
    exp_op = _register_exp16()
    exp_s0, exp_s1 = _exp16_consts(EXP_SCALE)
    Ident = mybir.ActivationFunctionType.Identity
    CopyF = mybir.ActivationFunctionType.Copy

    nc = bacc.Bacc("TRN2", target_bir_lowering=False, debug=False,
                   num_devices=N_CORES)

    xt_p = nc.declare_dram_parameter("xt", [D, NT], BF16, isOutput=False)
    wv_p = nc.declare_dram_parameter("wv", [128, D], BF16, isOutput=False)
    wo_p = nc.declare_dram_parameter("wo", [128, D], BF16, isOutput=False)
    bq_p = nc.declare_dram_parameter("bq", [128, 1], F32, isOutput=False)
    bk_p = nc.declare_dram_parameter("bk", [128, 1], F32, isOutput=False)
    bv_p = nc.declare_dram_parameter("bv", [128, 1], F32, isOutput=False)
    if QK_DR:
        xt8_p = nc.declare_dram_parameter("xt8", [D, NT], F8, isOutput=False)
        wq_p = nc.declare_dram_parameter("wq", [128, D], F8, isOutput=False)
        wk_p = nc.declare_dram_parameter("wk", [128, D], F8, isOutput=False)
    else:
        wq_p = nc.declare_dram_parameter("wq", [128, D], BF16, isOutput=False)
        wk_p = nc.declare_dram_parameter("wk", [128, D], BF16, isOutput=False)
    out16_p = nc.declare_dram_parameter("out16", [NT, D], BF16, isOutput=True)

    WDT = F8 if QK_DR else BF16

    with tile.TileContext(nc) as tc:
        with (
            tc.tile_pool(name="consts", bufs=1) as consts,
            tc.tile_pool(name="xts", bufs=3) as xts_pool,
            tc.tile_pool(name="qkv", bufs=2) as qkv_pool,
            tc.tile_pool(name="vtrp", bufs=4) as vtr_pool,
            tc.tile_pool(name="expp", bufs=4) as exp_pool,
            tc.tile_pool(name="attnp", bufs=3) as attn_pool,
            tc.tile_pool(name="small", bufs=3) as small,
            tc.tile_pool(name="outp", bufs=3) as out_pool,
            tc.tile_pool(name="ps_sp", bufs=2, space=bass.MemorySpace.PSUM) as ps_sp,
            tc.tile_pool(name="ps_av", bufs=2, space=bass.MemorySpace.PSUM) as ps_av,
            tc.tile_pool(name="ps_pp", bufs=1, space=bass.MemorySpace.PSUM) as ps_pp,
            tc.tile_pool(name="ps_po", bufs=1, space=bass.MemorySpace.PSUM) as ps_po,
        ):
            wq = consts.tile([128, D], WDT, name="wq")
            wk = consts.tile([128, D], WDT, name="wk")
            wv = consts.tile([128, D], BF16, name="wv")
            wo = consts.tile([128, D], BF16, name="wo")
            bq = consts.tile([128, 1], F32, name="bq")
            bk = consts.tile([128, 1], F32, name="bk")
            bv = consts.tile([128, 1], F32, name="bv")
            nc.sync.dma_start(wq[:], wq_p[:])
            nc.sync.dma_start(wk[:], wk_p[:])
            nc.sync.dma_start(wv[:], wv_p[:])
            nc.sync.dma_start(wo[:], wo_p[:])
            nc.sync.dma_start(bq[:], bq_p[:])
            nc.sync.dma_start(bk[:], bk_p[:])
            nc.sync.dma_start(bv[:], bv_p[:])

            # per-batch, per-block persistent tiles, allocated lazily
            qt = {}          # qt[b][tb] -> [128, 512]
            kt = {}          # kt[b][tb] -> [128, 512] (4 k-tiles each)
            vts = {}         # vts[b][tb] -> [128, 512]
            vaug = {}        # vaug[b][tb] -> [128, 4*VG] fp8
            xts_t = {}       # prefetched xT stacks (bf16, fp8)

            def _alloc_batch(b):
                qt[b] = [qkv_pool.tile([128, 512], BF16, tag=f"qt{t}",
                                       name=f"qt{b}_{t}") for t in range(4)]
                kt[b] = [qkv_pool.tile([128, 512], BF16, tag=f"kt{t}",
                                       name=f"kt{b}_{t}") for t in range(4)]
                vts[b] = [qkv_pool.tile([128, 512], BF16, tag=f"vts{t}",
                                        name=f"vts{b}_{t}") for t in range(4)]
                vaug[b] = []
                for t in range(4):
                    va = qkv_pool.tile([128, 4 * VG], F8, tag=f"vaug{t}",
                                       name=f"vaug{b}_{t}")
                    va3 = va.rearrange("p (k c) -> p k c", c=VG)
                    nc.gpsimd.memset(va3[:, :, 64:65], 1.0)
                    nc.gpsimd.memset(va3[:, :, HOFF + 64:HOFF + 65], 1.0)
                    vaug[b].append(va)

            def emit_proj_dma(b, tb):
                """Prefetch the 512-token xT stack(s) for block (b, tb)."""
                if b not in qt:
                    _alloc_batch(b)
                tok0 = b * S + tb * 512
                xts = xts_pool.tile([128, 8 * 512], BF16, tag="xts",
                                    name=f"xts{b}_{tb}")
                src3 = xt_p.rearrange("(kd p) t -> p kd t", p=128)
                dst3 = xts.rearrange("p (kd t) -> p kd t", t=512)
                nc.sync.dma_start(dst3[:, :, :], src3[:, :, tok0:tok0 + 512])
                if QK_DR:
                    xts8 = xts_pool.tile([128, 8 * 512], F8, tag="xts8",
                                         name=f"xts8{b}_{tb}")
                    src83 = xt8_p.rearrange("(kd p) t -> p kd t", p=128)
                    dst83 = xts8.rearrange("p (kd t) -> p kd t", t=512)
                    nc.sync.dma_start(dst83[:, :, :], src83[:, :, tok0:tok0 + 512])
                    xts_t[(b, tb)] = (xts, xts8)
                else:
                    xts_t[(b, tb)] = (xts, xts)

            def emit_proj_compute(b, tb):
                for clo in proj_closures(b, tb):
                    clo()

            def proj_closures(b, tb):
                """The projection block as a list of small closures so its PE
                work can be interleaved between attention score chunks.
                V runs first so its rotation DMAs are in flight long before
                the Pool quantize copies (emitted last) need them."""
                xts, xts8 = xts_t.pop((b, tb))

                # Build producer (PE matmul) and consumer (ACT bias / Pool
                # quantize) closures, then interleave so every consumer
                # drains a few slots after its producer — an in-order
                # engine never head-of-line blocks on a PE matmul.
                ppk = ps_pp.tile([128, 512], F32, tag="pp", name=f"ppk{b}_{tb}")
                ppq = ps_pp.tile([128, 512], F32, tag="pp", name=f"ppq{b}_{tb}")
                ppv = ps_pp.tile([128, 512], F32, tag="pp", name=f"ppv{b}_{tb}")

                def mk_qk(pp, w_sb):
                    if QK_DR:
                        w3 = w_sb.rearrange("p (k m) -> p k m", m=128)
                        x3 = xts8.rearrange("p (k t) -> p k t", t=512)

                        def mmdr(p0):
                            for p in (p0, p0 + 1):
                                nc.tensor.matmul(
                                    pp[:], w3[:, 2 * p:2 * p + 2, :],
                                    x3[:, 2 * p:2 * p + 2, :],
                                    start=(p == 0), stop=(p == 3),
                                    perf_mode=mybir.MatmulPerfMode.DoubleRow)
                        return [lambda p0=p0: mmdr(p0) for p0 in (0, 2)]

                    def mm2(kd0):
                        for kd in (kd0, kd0 + 1):
                            nc.tensor.matmul(
                                pp[:], w_sb[:, 128 * kd:128 * kd + 128],
                                xts[:, 512 * kd:512 * kd + 512],
                                start=(kd == 0), stop=(kd == 7))
                    return [lambda kd0=kd0: mm2(kd0) for kd0 in (0, 2, 4, 6)]

                def mm2v(kd0):
                    for kd in (kd0, kd0 + 1):
                        nc.tensor.matmul(
                            ppv[:], wv[:, 128 * kd:128 * kd + 128],
                            xts[:, 512 * kd:512 * kd + 512],
                            start=(kd == 0), stop=(kd == 7))

                vtrs = [vtr_pool.tile([128, 128], BF16, tag=f"vtr{ti}",
                                      name=f"vtr{b}_{tb}_{ti}")
                        for ti in range(4)]

                def rot_dma():
                    for ti in range(4):
                        nc.sync.dma_start_transpose(
                            vtrs[ti][:], vts[b][tb][:, 128 * ti:128 * ti + 128])

                va4 = vaug[b][tb].rearrange("p (k g e) -> p k g e", g=2, e=HOFF)

                def rot_q(ti):
                    vtr3 = vtrs[ti].rearrange("p (g e) -> p g e", e=64)
                    nc.gpsimd.tensor_copy(va4[:, ti, :, 0:64], vtr3[:, :, :])

                kbias = lambda: nc.scalar.activation(kt[b][tb][:], ppk[:],
                                                     Ident, bias=bk[:])
                qbias = lambda: nc.scalar.activation(qt[b][tb][:], ppq[:],
                                                     Ident, bias=bq[:])
                vbias = lambda: nc.scalar.activation(vts[b][tb][:], ppv[:],
                                                     Ident, bias=bv[:])
                kmm = mk_qk(ppk, wk)
                qmm = mk_qk(ppq, wq)
                vmm = [lambda kd0=kd0: mm2v(kd0) for kd0 in (0, 2, 4, 6)]
                return (kmm + [kbias] + qmm + [qbias] + vmm
                        + [vbias, rot_dma]
                        + [lambda ti=ti: rot_q(ti) for ti in range(4)])

            def outproj_closures(b, qb, attn):
                """Interleaved [mm, mm, copy, mm, copy, ...] so each
                PSUM->SBUF copy drains ~2 slots after its matmul."""
                obs = [out_pool.tile([128, 1024], BF16, tag="ob",
                                     name=f"ob{b}_{qb}_{ti}")
                       for ti in range(4)]
                pos = {}

                def op_mm(ti, j):
                    po = ps_po.tile([128, 512], F32, tag="po",
                                    name=f"po{b}_{qb}_{ti}_{j}")
                    nc.tensor.matmul(po[:],
                                     attn[:, 128 * ti:128 * ti + 128],
                                     wo[:, 512 * j:512 * j + 512],
                                     start=True, stop=True)
                    pos[(ti, j)] = po

                def op_cp(ti, j, on_act):
                    po = pos.pop((ti, j))
                    ob = obs[ti]
                    if on_act:
                        nc.scalar.activation(ob[:, 512 * j:512 * j + 512],
                                             po[:], CopyF)
                    else:
                        nc.vector.tensor_copy(ob[:, 512 * j:512 * j + 512],
                                              po[:])
                    if j == 1:
                        row0 = b * S + 512 * qb + 128 * ti
                        nc.sync.dma_start(out16_p[row0:row0 + 128, :], ob[:])

                halves = [(ti, j) for ti in range(4) for j in range(2)]
                mms = [lambda ti=ti, j=j: op_mm(ti, j) for ti, j in halves]
                cps = [lambda ti=ti, j=j, on_act=((ti + j) % 2 == 0):
                       op_cp(ti, j, on_act) for ti, j in halves]
                return mms, cps

            TRAIL = TRAIL_G

            class Unit:
                """Per-unit attention state for the continuous chunk stream."""

                def __init__(self, b, qb):
                    self.b, self.qb = b, qb
                    self.av = None
                    self.ecs = {}

                def emit_scores_exp(self, c):
                    b, qb = self.b, self.qb
                    qsl = qt[b][qb]
                    tb_of_c = c // 2        # source projection block
                    cc = c % 2              # k-tile pair within block
                    for h in range(2):
                        ec = exp_pool.tile([128, 1024], F8, tag=f"expt{h}",
                                           bufs=TRAIL_G + 2,
                                           name=f"ec{b}_{qb}_{c}_{h}")
                        if SPLIT_EXP:
                            for j in range(2):
                                sp = ps_sp.tile([128, 512], F32, tag="sp",
                                                bufs=SP_BUFS,
                                                name=f"sp{b}_{qb}_{c}_{h}_{j}")
                                kt_loc = 2 * cc + j
                                nc.tensor.matmul(
                                    sp[:],
                                    kt[b][tb_of_c][64 * h:64 * h + 64,
                                                   128 * kt_loc:128 * kt_loc + 128],
                                    qsl[64 * h:64 * h + 64, :],
                                    start=True, stop=True)
                                ech = ec[:, 512 * j:512 * j + 512]
                                if (c, h) in DVE_EXP:
                                    nc.vector._custom_dve(exp_op, out=ech,
                                                          in0=sp[:],
                                                          s0=exp_s0, s1=exp_s1)
                                else:
                                    nc.scalar.activation(
                                        ech, sp[:],
                                        mybir.ActivationFunctionType.Exp,
                                        scale=EXP_SCALE)
                        else:
                            sp = ps_sp.tile([128, 1024], F32, tag="sp", bufs=2,
                                            name=f"sp{b}_{qb}_{c}_{h}")
                            for j in range(2):
                                kt_loc = 2 * cc + j
                                nc.tensor.matmul(
                                    sp[:, 512 * j:512 * j + 512],
                                    kt[b][tb_of_c][64 * h:64 * h + 64,
                                                   128 * kt_loc:128 * kt_loc + 128],
                                    qsl[64 * h:64 * h + 64, :],
                                    start=True, stop=True)
                            if (c, h) in DVE_EXP:
                                nc.vector._custom_dve(exp_op, out=ec[:],
                                                      in0=sp[:],
                                                      s0=exp_s0, s1=exp_s1)
                            else:
                                nc.scalar.activation(
                                    ec[:], sp[:],
                                    mybir.ActivationFunctionType.Exp,
                                    scale=EXP_SCALE)
                        self.ecs[(c, h)] = ec

                def emit_av(self, cp):
                    b, qb = self.b, self.qb
                    if self.av is None:
                        self.av = [ps_av.tile([65, 512], F32, tag="av",
                                              name=f"av{b}_{qb}_{h}")
                                   for h in range(2)]
                    tb_p = cp // 2
                    ccp = cp % 2
                    va3 = vaug[b][tb_p].rearrange("p (k c) -> p k c", c=VG)
                    for h in range(2):
                        ec3 = self.ecs.pop((cp, h)).rearrange(
                            "p (k q) -> p k q", q=512)
                        nc.tensor.matmul(
                            self.av[h][:],
                            va3[:, 2 * ccp:2 * ccp + 2, HOFF * h:HOFF * h + 65],
                            ec3[:, :, :],
                            start=(cp == 0), stop=(cp == NCH - 1),
                            perf_mode=mybir.MatmulPerfMode.DoubleRow)

                def emit_normalize(self):
                    b, qb = self.b, self.qb
                    attn = attn_pool.tile([128, 512], BF16, tag="attn",
                                          name=f"attn{b}_{qb}")
                    rrows, bcs = [], []
                    for h in range(2):
                        rrow = small.tile([1, 512], F32, tag=f"rrow{h}",
                                          name=f"rr{b}_{qb}_{h}")
                        nc.vector.reciprocal(rrow[:], self.av[h][64:65, :])
                        rrows.append(rrow)
                    for h in range(2):
                        bc_sb = small.tile([64, 512], F32, tag=f"bc_sb{h}",
                                           name=f"bs{b}_{qb}_{h}")
                        nc.gpsimd.partition_broadcast(bc_sb[:], rrows[h])
                        bcs.append(bc_sb)
                    for h in range(2):
                        nc.vector.tensor_mul(attn[64 * h:64 * h + 64, :],
                                             self.av[h][0:64, :], bcs[h])
                    return attn

            # Continuous chunk stream: unit u's chunks 0..7 emit scores+exp;
            # its AV matmuls trail by TRAIL chunks, spilling into unit u+1's
            # first TRAIL chunks; normalize for unit u is emitted right after
            # its last AV (chunk TRAIL of unit u+1), and its out-proj drains
            # later in that unit via the bg queue. Projection blocks for
            # batch b+1 interleave throughout. No engine ever runs dry at a
            # unit boundary.
            from collections import deque
            bgA = deque()        # projection closures (internally ordered)
            bgM = deque()        # out-proj matmuls (producers)
            bgC = deque()        # out-proj PSUM->SBUF copies (consumers)
            nM = nC = 0          # popped counts for producer/consumer pacing
            units = [(b, qb) for b in range(B) for qb in range(QB_PER_B)]

            def drain_mc():
                nonlocal nM, nC
                # po pool is single-buffered: the copy of matmul i must be
                # emitted before matmul i+1 (WAR on the recycled bank)
                if bgC and (nM - nC >= 1 or not bgM):
                    bgC.popleft()()
                    nC += 1
                elif bgM:
                    bgM.popleft()()
                    nM += 1

            for tb in range(TB_PER_B):
                emit_proj_dma(0, tb)
            emit_proj_compute(0, 0)
            emit_proj_compute(0, 1)
            bgA.extend(proj_closures(0, 2))
            bgA.extend(proj_closures(0, 3))

            prev = None          # unit with AV trail / normalize outstanding
            pend_out = None      # out-proj closures awaiting queue insertion
            for u, (b, qb) in enumerate(units):
                cur = Unit(b, qb)
                carry = 0
                if b + 1 < B:
                    emit_proj_dma(b + 1, qb)
                    newc = proj_closures(b + 1, qb)
                    bgA.extend(newc)
                    # the block added this unit may spill into the next unit:
                    # its kt/vaug are first read >=1 unit later
                    carry = len(newc)
                if pend_out is not None:
                    bgM.extend(pend_out[0])
                    bgC.extend(pend_out[1])
                    pend_out = None
                for c in range(NCH):
                    cur.emit_scores_exp(c)
                    if c >= TRAIL:
                        cur.emit_av(c - TRAIL)
                    elif prev is not None:
                        prev.emit_av(NCH - TRAIL + c)
                        if c == TRAIL - 1:
                            attn = prev.emit_normalize()
                            pend_out = outproj_closures(prev.b, prev.qb, attn)
                    # pace the projection queue so it fully drains within
                    # this unit: chunk c of NCH has (NCH - c) chunks left,
                    # and the next unit's trailing AVs / scores depend on
                    # this unit's projection outputs (kt, vaug)
                    na = max(2, -(-max(0, len(bgA) - carry) // (NCH - c)))
                    for _ in range(na):
                        if bgA:
                            bgA.popleft()()
                    for _ in range(MC_POPS):
                        drain_mc()
                prev = cur
            for c in range(TRAIL):
                prev.emit_av(NCH - TRAIL + c)
            attn = prev.emit_normalize()
            if pend_out is not None:
                bgM.extend(pend_out[0])
                bgC.extend(pend_out[1])
            mo, co = outproj_closures(prev.b, prev.qb, attn)
            bgM.extend(mo)
            bgC.extend(co)
            while bgA or bgM or bgC:
                if bgA:
                    bgA.popleft()()
                drain_mc()

    nc.compile()
    return nc


def _get_compiled():
    global _COMPILED
    if _COMPILED is None:
        _COMPILED = _build()
    return _COMPILED


def _prep_inputs(x, Wq, bq, Wk, bk, Wv, bv, Wo, bo):
    xf = np.asarray(x, dtype=np.float32).reshape(NT, D).T
    xt = np.ascontiguousarray(xf).astype(bf16)

    def pack_w(Wc, dtype, scale=1.0):
        # [128 out, 1024 in] -> k-tile packed [128, 1024]
        wt = np.asarray(Wc, dtype=np.float32).T * scale  # [1024 in, 128 out]
        return np.ascontiguousarray(
            wt.reshape(8, 128, 128).transpose(1, 0, 2).reshape(128, D)).astype(dtype)

    if QK_DR:
        xt8 = np.ascontiguousarray(xf).astype(f8)

    in_maps = []
    for c in range(N_CORES):
        sl = slice(128 * c, 128 * c + 128)
        m = {
            "xt": xt,
            "wv": pack_w(np.asarray(Wv)[sl], bf16),
            "wo": np.ascontiguousarray(
                np.asarray(Wo, dtype=np.float32)[:, sl].T).astype(bf16),
            "bv": np.asarray(bv, dtype=np.float32)[sl].reshape(128, 1),
        }
        if QK_DR:
            m["xt8"] = xt8
            m["wq"] = pack_w(np.asarray(Wq)[sl], f8, QK_W_SCALE)
            m["wk"] = pack_w(np.asarray(Wk)[sl], f8, QK_W_SCALE)
            m["bq"] = (np.asarray(bq, dtype=np.float32)[sl] *
                       QK_W_SCALE).reshape(128, 1)
            m["bk"] = (np.asarray(bk, dtype=np.float32)[sl] *
                       QK_W_SCALE).reshape(128, 1)
        else:
            m["wq"] = pack_w(np.asarray(Wq)[sl], bf16)
            m["wk"] = pack_w(np.asarray(Wk)[sl], bf16)
            m["bq"] = np.asarray(bq, dtype=np.float32)[sl].reshape(128, 1)
            m["bk"] = np.asarray(bk, dtype=np.float32)[sl].reshape(128, 1)
        in_maps.append(m)
    return in_maps


def kernel(x, Wq, bq, Wk, bk, Wv, bv, Wo, bo):
    nc = _get_compiled()
    in_maps = _prep_inputs(x, Wq, bq, Wk, bk, Wv, bv, Wo, bo)
    res = run_bass_kernel_spmd(nc, in_maps, core_ids=list(range(N_CORES)))
    acc = np.zeros((NT, D), dtype=np.float32)
    for c in range(N_CORES):
        acc += np.asarray(res.results[c]["out16"]).astype(np.float32)
    acc += np.asarray(bo, dtype=np.float32)[None, :]
    return acc.reshape(B, S, D)


# revision 37
# speedup vs baseline: 1.0599x; 1.0210x over previous
"""MultiHeadAttention Trainium2 kernel, 8-way tensor-parallel by head.

Problem: B=4, S=2048, D=1024, 16 heads, d_k=64 (nn_MultiHeadAttention_67585605370071).

Sharding: each core owns 2 heads (128 of the 1024 hidden dims):
  Wq/Wk/Wv column-sharded by head, Wo row-sharded; the 8 partial outputs
  are summed on the host (the row-shard reduction) and bo is added there.

Dataflow per core (matmuls bf16 / fp8-DoubleRow, f32 PSUM accumulation):
  - host passes x pre-transposed (xT bf16 [1024, 8192]); Q/K/V projections
    run bf16 (QK_DR=True switches Q/K to fp8 DoubleRow for ~4us, at the
    cost of rel-err 1.5e-2 -> 2.0e-2)
  - V rotated to token-major layout via SBUF->SBUF DMA transposes, then
    quantized fp8 into the augmented layout [V_A | 1 | pad | V_B | 1 | pad]
    by the GpSimd engine; the ones column makes the AV matmul emit softmax
    denominators as row 64 of its PSUM tile
  - scoresT = Kt.T @ Qt per [128,512] half-tile; exp is split between the
    ScalarE ACT (table exp) and a custom DVE microcode op EXP16_POLY_ANT
    ((1 + ks + (ks)^2/2)^16, 8/8 uop stages) so neither engine saturates;
    both emit fp8
  - AV via fp8 DoubleRow matmuls (2 k-tiles per pass), trailing the
    scores/exp stream by TRAIL chunks so the PE never stalls on exp latency
  - normalize via DVE reciprocal + GpSimd partition_broadcast + DVE
    tensor_mul (no PE broadcast matmul)
  - out-proj per 128-token tile; PSUM->SBUF copies alternate between an ACT
    Copy and a DVE tensor_copy; projection bias-adds run as ACT
    Identity+bias (exp/identity/copy share one ACT table - no reloads)
  - emission is a continuous chunk stream: unit u's trailing AVs and
    normalize ride in unit u+1's first chunks, projection closures drain
    with ceiling pacing (fully inside their unit - the next unit's scores
    and trailing AVs depend on kt/vaug), and out-proj matmul/copy pairs
    drain through paced producer/consumer queues that keep the single
    po PSUM bank's write-after-read order identical to emission order
PSUM budget (8 banks): 4x score [128,512] + 2x AV accumulators [65,512]
  + 1 projection bank + 1 out-proj bank.
"""
import numpy as np
import ml_dtypes

import concourse.bass as bass
import concourse.bacc as bacc
import concourse.mybir as mybir
import concourse.tile as tile
from concourse.bass_utils import run_bass_kernel_spmd

import concourse.dve_ops as dve_ops_mod
from concourse.dve_spec import C0, C1, One, Spec, Src0, lower as dve_lower, sq
from concourse.dve_uop import DveOpSpec

BF16 = mybir.dt.bfloat16
F32 = mybir.dt.float32
F8 = mybir.dt.float8e4
bf16 = ml_dtypes.bfloat16
f8 = ml_dtypes.float8_e4m3

B, S, D = 4, 2048, 1024
NT = B * S            # 8192 tokens
N_CORES = 8
KT_PER_B = S // 128   # 16 k-tiles per batch
QB_PER_B = S // 512   # 4 query blocks per batch
TB_PER_B = S // 512   # 4 token blocks per batch (projection)
NCH = KT_PER_B // 2   # 8 score/exp chunks per attention unit

# fp8-e4m3 attention-weights path: expT and V quantized to fp8 so the AV
# matmul can use DoubleRow (2x PE throughput).
VG = 144              # vaug per-k-tile column group
HOFF = 72             # head B column offset inside a group

# Q/K projections in fp8 DoubleRow; weights scaled by 32 on the host, the
# 32*32 factor folded into the exp scale.
QK_DR = False
QK_W_SCALE = 32.0
EXP_SCALE = 0.125 / (QK_W_SCALE * QK_W_SCALE if QK_DR else 1.0)

# exp tiles handed to the DVE custom op instead of ScalarE, per unit
DVE_EXP = {(c, 1) for c in range(7)}
SPLIT_EXP = True     # exp per [128,512] half (4 sp bufs) vs full [128,1024]
SPLIT_OUTPROJ = False  # out-proj mm/copy as separate bg closures
NDRAIN = 3           # bg closures drained per chunk
SP_BUFS = 4          # score-PSUM buffers ([128,512] each)
TRAIL_G = 5          # chunks the AV stream trails scores/exp by
MC_POPS = 2          # out-proj producer/consumer pops per chunk
SH_BUFS = 2          # shared pp/po PSUM buffers

_COMPILED = None

# ---------------------------------------------------------------------------
# Custom DVE exp op: out = exp(in * scale) via (1 + ks + (ks)^2/2)^16,
# k = scale/16 — quadratic (4 ALU stages) + 4 squarings = 8/8 v3 stages.
# Rel err vs true exp: rms ~1e-4; after fp8 output quantization 99.96% of
# outputs are bit-identical to fp8(exp(x)).
EXP16_NAME = "EXP16_POLY_ANT"


def _exp16_consts(scale):
    k = scale / 16.0
    return (k * k / 2.0, k)


def _exp16_ref(in0, in1, s0, s1, imm2):
    b = ((in0.astype(np.float32) * s0 + s1) * in0 + 1.0).astype(np.float32)
    for _ in range(4):
        b = (b * b).astype(np.float32)
    return b


def _register_exp16():
    for op in dve_ops_mod.OPS:
        if op.name == EXP16_NAME:
            return op
    body = sq(sq(sq(sq((Src0 * C0 + C1) * Src0 + One))))
    spec = Spec(body=body, reference=_exp16_ref)
    row = max(dve_ops_mod._SUB_OPCODE_FOR_NAME.values()) + 1
    assert row < 0x20, "custom-DVE opcode rows exhausted"
    dve_ops_mod._SUB_OPCODE_FOR_NAME[EXP16_NAME] = row
    shas = {}
    for ver in ("v3", "v4"):
        s = DveOpSpec(name=EXP16_NAME, opcode=row,
                      uops=dve_lower(spec, ver=ver), rd1_en=False)
        shas[ver] = s.sha(ver)
    op = dve_ops_mod.DveOp(EXP16_NAME, spec, False, shas)
    dve_ops_mod.OPS.append(op)
    dve_ops_mod.CUSTOM_DVE_SPECS[EXP16_NAME] = spec
    return op


# ---------------------------------------------------------------------------
def _build():
    exp_op = _register_exp16()
    exp_s0, exp_s1 = _exp16_consts(EXP_SCALE)
    Ident = mybir.ActivationFunctionType.Identity
    CopyF = mybir.ActivationFunctionType.Copy

    nc = bacc.Bacc("TRN2", target_bir_lowering=False, debug=False,
                   num_devices=N_CORES)

    xt_p = nc.declare_dram_parameter("xt", [D, NT], BF16, isOutput=False)
    wv_p = nc.declare_dram_parameter("wv", [128, D], BF16, isOutput=False)
    wo_p = nc.declare_dram_parameter("wo", [128, D], BF16, isOutput=False)
    bq_p = nc.declare_dram_parameter("bq", [128, 1], F32, isOutput=False)
    bk_p = nc.declare_dram_parameter("bk", [128, 1], F32, isOutput=False)
    bv_p = nc.declare_dram_parameter("bv", [128, 1], F32, isOutput=False)
    if QK_DR:
        xt8_p = nc.declare_dram_parameter("xt8", [D, NT], F8, isOutput=False)
        wq_p = nc.declare_dram_parameter("wq", [128, D], F8, isOutput=False)
        wk_p = nc.declare_dram_parameter("wk", [128, D], F8, isOutput=False)
    else:
        wq_p = nc.declare_dram_parameter("wq", [128, D], BF16, isOutput=False)
        wk_p = nc.declare_dram_parameter("wk", [128, D], BF16, isOutput=False)
    out16_p = nc.declare_dram_parameter("out16", [NT, D], BF16, isOutput=True)

    WDT = F8 if QK_DR else BF16

    with tile.TileContext(nc) as tc:
        with (
            tc.tile_pool(name="consts", bufs=1) as consts,
            tc.tile_pool(name="xts", bufs=3) as xts_pool,
            tc.tile_pool(name="qkv", bufs=2) as qkv_pool,
            tc.tile_pool(name="vtrp", bufs=4) as vtr_pool,
            tc.tile_pool(name="expp", bufs=4) as exp_pool,
            tc.tile_pool(name="attnp", bufs=3) as attn_pool,
            tc.tile_pool(name="small", bufs=3) as small,
            tc.tile_pool(name="outp", bufs=3) as out_pool,
            tc.tile_pool(name="ps_sp", bufs=2, space=bass.MemorySpace.PSUM) as ps_sp,
            tc.tile_pool(name="ps_av", bufs=2, space=bass.MemorySpace.PSUM) as ps_av,
            tc.tile_pool(name="ps_pp", bufs=1, space=bass.MemorySpace.PSUM) as ps_pp,
            tc.tile_pool(name="ps_po", bufs=1, space=bass.MemorySpace.PSUM) as ps_po,
        ):
            wq = consts.tile([128, D], WDT, name="wq")
            wk = consts.tile([128, D], WDT, name="wk")
            wv = consts.tile([128, D], BF16, name="wv")
            wo = consts.tile([128, D], BF16, name="wo")
            bq = consts.tile([128, 1], F32, name="bq")
            bk = consts.tile([128, 1], F32, name="bk")
            bv = consts.tile([128, 1], F32, name="bv")
            nc.sync.dma_start(wq[:], wq_p[:])
            nc.sync.dma_start(wk[:], wk_p[:])
            nc.sync.dma_start(wv[:], wv_p[:])
            nc.sync.dma_start(wo[:], wo_p[:])
            nc.sync.dma_start(bq[:], bq_p[:])
            nc.sync.dma_start(bk[:], bk_p[:])
            nc.sync.dma_start(bv[:], bv_p[:])

            # per-batch, per-block persistent tiles, allocated lazily
            qt = {}          # qt[b][tb] -> [128, 512]
            kt = {}          # kt[b][tb] -> [128, 512] (4 k-tiles each)
            vts = {}         # vts[b][tb] -> [128, 512]
            vaug = {}        # vaug[b][tb] -> [128, 4*VG] fp8
            xts_t = {}       # prefetched xT stacks (bf16, fp8)

            def _alloc_batch(b):
                qt[b] = [qkv_pool.tile([128, 512], BF16, tag=f"qt{t}",
                                       name=f"qt{b}_{t}") for t in range(4)]
                kt[b] = [qkv_pool.tile([128, 512], BF16, tag=f"kt{t}",
                                       name=f"kt{b}_{t}") for t in range(4)]
                vts[b] = [qkv_pool.tile([128, 512], BF16, tag=f"vts{t}",
                                        name=f"vts{b}_{t}") for t in range(4)]
                vaug[b] = []
                for t in range(4):
                    va = qkv_pool.tile([128, 4 * VG], F8, tag=f"vaug{t}",
                                       name=f"vaug{b}_{t}")
                    va3 = va.rearrange("p (k c) -> p k c", c=VG)
                    nc.gpsimd.memset(va3[:, :, 64:65], 1.0)
                    nc.gpsimd.memset(va3[:, :, HOFF + 64:HOFF + 65], 1.0)
                    vaug[b].append(va)

            def emit_proj_dma(b, tb):
                """Prefetch the 512-token xT stack(s) for block (b, tb)."""
                if b not in qt:
                    _alloc_batch(b)
                tok0 = b * S + tb * 512
                xts = xts_pool.tile([128, 8 * 512], BF16, tag="xts",
                                    name=f"xts{b}_{tb}")
                src3 = xt_p.rearrange("(kd p) t -> p kd t", p=128)
                dst3 = xts.rearrange("p (kd t) -> p kd t", t=512)
                nc.sync.dma_start(dst3[:, :, :], src3[:, :, tok0:tok0 + 512])
                if QK_DR:
                    xts8 = xts_pool.tile([128, 8 * 512], F8, tag="xts8",
                                         name=f"xts8{b}_{tb}")
                    src83 = xt8_p.rearrange("(kd p) t -> p kd t", p=128)
                    dst83 = xts8.rearrange("p (kd t) -> p kd t", t=512)
                    nc.sync.dma_start(dst83[:, :, :], src83[:, :, tok0:tok0 + 512])
                    xts_t[(b, tb)] = (xts, xts8)
                else:
                    xts_t[(b, tb)] = (xts, xts)

            def emit_proj_compute(b, tb):
                for clo in proj_closures(b, tb):
                    clo()

            def proj_closures(b, tb):
                """The projection block as a list of small closures so its PE
                work can be interleaved between attention score chunks.
                V runs first so its rotation DMAs are in flight long before
                the Pool quantize copies (emitted last) need them."""
                xts, xts8 = xts_t.pop((b, tb))

                # Build producer (PE matmul) and consumer (ACT bias / Pool
                # quantize) closures, then interleave so every consumer
                # drains a few slots after its producer — an in-order
                # engine never head-of-line blocks on a PE matmul.
                ppk = ps_pp.tile([128, 512], F32, tag="pp", name=f"ppk{b}_{tb}")
                ppq = ps_pp.tile([128, 512], F32, tag="pp", name=f"ppq{b}_{tb}")
                ppv = ps_pp.tile([128, 512], F32, tag="pp", name=f"ppv{b}_{tb}")

                def mk_qk(pp, w_sb):
                    if QK_DR:
                        w3 = w_sb.rearrange("p (k m) -> p k m", m=128)
                        x3 = xts8.rearrange("p (k t) -> p k t", t=512)

                        def mmdr(p0):
                            for p in (p0, p0 + 1):
                                nc.tensor.matmul(
                                    pp[:], w3[:, 2 * p:2 * p + 2, :],
                                    x3[:, 2 * p:2 * p + 2, :],
                                    start=(p == 0), stop=(p == 3),
                                    perf_mode=mybir.MatmulPerfMode.DoubleRow)
                        return [lambda p0=p0: mmdr(p0) for p0 in (0, 2)]

                    def mm2(kd0):
                        for kd in (kd0, kd0 + 1):
                            nc.tensor.matmul(
                                pp[:], w_sb[:, 128 * kd:128 * kd + 128],
                                xts[:, 512 * kd:512 * kd + 512],
                                start=(kd == 0), stop=(kd == 7))
                    return [lambda kd0=kd0: mm2(kd0) for kd0 in (0, 2, 4, 6)]

                def mm2v(kd0):
                    for kd in (kd0, kd0 + 1):
                        nc.tensor.matmul(
                            ppv[:], wv[:, 128 * kd:128 * kd + 128],
                            xts[:, 512 * kd:512 * kd + 512],
                            start=(kd == 0), stop=(kd == 7))

                vtrs = [vtr_pool.tile([128, 128], BF16, tag=f"vtr{ti}",
                                      name=f"vtr{b}_{tb}_{ti}")
                        for ti in range(4)]

                def rot_dma():
                    for ti in range(4):
                        nc.sync.dma_start_transpose(
                            vtrs[ti][:], vts[b][tb][:, 128 * ti:128 * ti + 128])

                va4 = vaug[b][tb].rearrange("p (k g e) -> p k g e", g=2, e=HOFF)

                def rot_q(ti):
                    vtr3 = vtrs[ti].rearrange("p (g e) -> p g e", e=64)
                    nc.gpsimd.tensor_copy(va4[:, ti, :, 0:64], vtr3[:, :, :])

                kbias = lambda: nc.scalar.activation(kt[b][tb][:], ppk[:],
                                                     Ident, bias=bk[:])
                qbias = lambda: nc.scalar.activation(qt[b][tb][:], ppq[:],
                                                     Ident, bias=bq[:])
                vbias = lambda: nc.scalar.activation(vts[b][tb][:], ppv[:],
                                                     Ident, bias=bv[:])
                kmm = mk_qk(ppk, wk)
                qmm = mk_qk(ppq, wq)
                vmm = [lambda kd0=kd0: mm2v(kd0) for kd0 in (0, 2, 4, 6)]
                return (kmm + [kbias] + qmm + [qbias] + vmm
                        + [vbias, rot_dma]
                        + [lambda ti=ti: rot_q(ti) for ti in range(4)])

            def outproj_closures(b, qb, attn):
                """Interleaved [mm, mm, copy, mm, copy, ...] so each
                PSUM->SBUF copy drains ~2 slots after its matmul."""
                obs = [out_pool.tile([128, 1024], BF16, tag="ob",
                                     name=f"ob{b}_{qb}_{ti}")
                       for ti in range(4)]
                pos = {}

                def op_mm(ti, j):
                    po = ps_po.tile([128, 512], F32, tag="po",
                                    name=f"po{b}_{qb}_{ti}_{j}")
                    nc.tensor.matmul(po[:],
                                     attn[:, 128 * ti:128 * ti + 128],
                                     wo[:, 512 * j:512 * j + 512],
                                     start=True, stop=True)
                    pos[(ti, j)] = po

                def op_cp(ti, j, on_act):
                    po = pos.pop((ti, j))
                    ob = obs[ti]
                    if on_act:
                        nc.scalar.activation(ob[:, 512 * j:512 * j + 512],
                                             po[:], CopyF)
                    else:
                        nc.vector.tensor_copy(ob[:, 512 * j:512 * j + 512],
                                              po[:])
                    if j == 1:
                        row0 = b * S + 512 * qb + 128 * ti
                        nc.sync.dma_start(out16_p[row0:row0 + 128, :], ob[:])

                halves = [(ti, j) for ti in range(4) for j in range(2)]
                mms = [lambda ti=ti, j=j: op_mm(ti, j) for ti, j in halves]
                cps = [lambda ti=ti, j=j, on_act=((ti + j) % 2 == 0):
                       op_cp(ti, j, on_act) for ti, j in halves]
                return mms, cps

            TRAIL = TRAIL_G

            class Unit:
                """Per-unit attention state for the continuous chunk stream."""

                def __init__(self, b, qb):
                    self.b, self.qb = b, qb
                    self.av = None
                    self.ecs = {}

                def emit_scores_exp(self, c):
                    b, qb = self.b, self.qb
                    qsl = qt[b][qb]
                    tb_of_c = c // 2        # source projection block
                    cc = c % 2              # k-tile pair within block
                    for h in range(2):
                        ec = exp_pool.tile([128, 1024], F8, tag=f"expt{h}",
                                           bufs=TRAIL_G + 2,
                                           name=f"ec{b}_{qb}_{c}_{h}")
                        if SPLIT_EXP:
                            for j in range(2):
                                sp = ps_sp.tile([128, 512], F32, tag="sp",
                                                bufs=SP_BUFS,
                                                name=f"sp{b}_{qb}_{c}_{h}_{j}")
                                kt_loc = 2 * cc + j
                                nc.tensor.matmul(
                                    sp[:],
                                    kt[b][tb_of_c][64 * h:64 * h + 64,
                                                   128 * kt_loc:128 * kt_loc + 128],
                                    qsl[64 * h:64 * h + 64, :],
                                    start=True, stop=True)
                                ech = ec[:, 512 * j:512 * j + 512]
                                if (c, h) in DVE_EXP:
                                    nc.vector._custom_dve(exp_op, out=ech,
                                                          in0=sp[:],
                                                          s0=exp_s0, s1=exp_s1)
                                else:
                                    nc.scalar.activation(
                                        ech, sp[:],
                                        mybir.ActivationFunctionType.Exp,
                                        scale=EXP_SCALE)
                        else:
                            sp = ps_sp.tile([128, 1024], F32, tag="sp", bufs=2,
                                            name=f"sp{b}_{qb}_{c}_{h}")
                            for j in range(2):
                                kt_loc = 2 * cc + j
                                nc.tensor.matmul(
                                    sp[:, 512 * j:512 * j + 512],
                                    kt[b][tb_of_c][64 * h:64 * h + 64,
                                                   128 * kt_loc:128 * kt_loc + 128],
                                    qsl[64 * h:64 * h + 64, :],
                                    start=True, stop=True)
                            if (c, h) in DVE_EXP:
                                nc.vector._custom_dve(exp_op, out=ec[:],
                                                      in0=sp[:],
                                                      s0=exp_s0, s1=exp_s1)
                            else:
                                nc.scalar.activation(
                                    ec[:], sp[:],
                                    mybir.ActivationFunctionType.Exp,
                                    scale=EXP_SCALE)
                        self.ecs[(c, h)] = ec

                def emit_av(self, cp):
                    b, qb = self.b, self.qb
                    if self.av is None:
                        self.av = [ps_av.tile([65, 512], F32, tag="av",
                                              name=f"av{b}_{qb}_{h}")
                                   for h in range(2)]
                    tb_p = cp // 2
                    ccp = cp % 2
                    va3 = vaug[b][tb_p].rearrange("p (k c) -> p k c", c=VG)
                    for h in range(2):
                        ec3 = self.ecs.pop((cp, h)).rearrange(
                            "p (k q) -> p k q", q=512)
                        nc.tensor.matmul(
                            self.av[h][:],
                            va3[:, 2 * ccp:2 * ccp + 2, HOFF * h:HOFF * h + 65],
                            ec3[:, :, :],
                            start=(cp == 0), stop=(cp == NCH - 1),
                            perf_mode=mybir.MatmulPerfMode.DoubleRow)

                def emit_normalize(self):
                    b, qb = self.b, self.qb
                    attn = attn_pool.tile([128, 512], BF16, tag="attn",
                                          name=f"attn{b}_{qb}")
                    rrows, bcs = [], []
                    for h in range(2):
                        rrow = small.tile([1, 512], F32, tag=f"rrow{h}",
                                          name=f"rr{b}_{qb}_{h}")
                        nc.vector.reciprocal(rrow[:], self.av[h][64:65, :])
                        rrows.append(rrow)
                    for h in range(2):
                        bc_sb = small.tile([64, 512], F32, tag=f"bc_sb{h}",
                                           name=f"bs{b}_{qb}_{h}")
                        nc.gpsimd.partition_broadcast(bc_sb[:], rrows[h])
                        bcs.append(bc_sb)
                    for h in range(2):
                        nc.vector.tensor_mul(attn[64 * h:64 * h + 64, :],
                                             self.av[h][0:64, :], bcs[h])
                    return attn

            # Continuous chunk stream: unit u's chunks 0..7 emit scores+exp;
            # its AV matmuls trail by TRAIL chunks, spilling into unit u+1's
            # first TRAIL chunks; normalize for unit u is emitted right after
            # its last AV (chunk TRAIL of unit u+1), and its out-proj drains
            # later in that unit via the bg queue. Projection blocks for
            # batch b+1 interleave throughout. No engine ever runs dry at a
            # unit boundary.
            from collections import deque
            bgA = deque()        # projection closures (internally ordered)
            bgM = deque()        # out-proj matmuls (producers)
            bgC = deque()        # out-proj PSUM->SBUF copies (consumers)
            nM = nC = 0          # popped counts for producer/consumer pacing
            units = [(b, qb) for b in range(B) for qb in range(QB_PER_B)]

            def drain_mc():
                nonlocal nM, nC
                # po pool is single-buffered: the copy of matmul i must be
                # emitted before matmul i+1 (WAR on the recycled bank)
                if bgC and (nM - nC >= 1 or not bgM):
                    bgC.popleft()()
                    nC += 1
                elif bgM:
                    bgM.popleft()()
                    nM += 1

            for tb in range(TB_PER_B):
                emit_proj_dma(0, tb)
            emit_proj_compute(0, 0)
            bgA.extend(proj_closures(0, 1))
            bgA.extend(proj_closures(0, 2))
            bgA.extend(proj_closures(0, 3))

            prev = None          # unit with AV trail / normalize outstanding
            pend_out = None      # out-proj closures awaiting queue insertion
            for u, (b, qb) in enumerate(units):
                cur = Unit(b, qb)
                carry = 0
                if b + 1 < B:
                    emit_proj_dma(b + 1, qb)
                    newc = proj_closures(b + 1, qb)
                    bgA.extend(newc)
                    # the block added this unit may spill into the next unit:
                    # its kt/vaug are first read >=1 unit later
                    carry = len(newc)
                if pend_out is not None:
                    bgM.extend(pend_out[0])
                    bgC.extend(pend_out[1])
                    pend_out = None
                for c in range(NCH):
                    cur.emit_scores_exp(c)
                    if c >= TRAIL:
                        cur.emit_av(c - TRAIL)
                    elif prev is not None:
                        prev.emit_av(NCH - TRAIL + c)
                        if c == TRAIL - 1:
                            attn = prev.emit_normalize()
                            pend_out = outproj_closures(prev.b, prev.qb, attn)
                    # pace the projection queue so it fully drains within
                    # this unit: chunk c of NCH has (NCH - c) chunks left,
                    # and the next unit's trailing AVs / scores depend on
                    # this unit's projection outputs (kt, vaug)
                    na = max(2, -(-max(0, len(bgA) - carry) // (NCH - c)))
                    for _ in range(na):
                        if bgA:
                            bgA.popleft()()
                    for _ in range(MC_POPS):
                        drain_mc()
                prev = cur
            for c in range(TRAIL):
                prev.emit_av(NCH - TRAIL + c)
            attn = prev.emit_normalize()
            if pend_out is not None:
                bgM.extend(pend_out[0])
                bgC.extend(pend_out[1])
            mo, co = outproj_closures(prev.b, prev.qb, attn)
            bgM.extend(mo)
            bgC.extend(co)
            while bgA or bgM or bgC:
                if bgA:
                    bgA.popleft()()
                drain_mc()

    nc.compile()
    return nc


def _get_compiled():
    global _COMPILED
    if _COMPILED is None:
        _COMPILED = _build()
    return _COMPILED


def _prep_inputs(x, Wq, bq, Wk, bk, Wv, bv, Wo, bo):
    xf = np.asarray(x, dtype=np.float32).reshape(NT, D).T
    xt = np.ascontiguousarray(xf).astype(bf16)

    def pack_w(Wc, dtype, scale=1.0):
        # [128 out, 1024 in] -> k-tile packed [128, 1024]
        wt = np.asarray(Wc, dtype=np.float32).T * scale  # [1024 in, 128 out]
        return np.ascontiguousarray(
            wt.reshape(8, 128, 128).transpose(1, 0, 2).reshape(128, D)).astype(dtype)

    if QK_DR:
        xt8 = np.ascontiguousarray(xf).astype(f8)

    in_maps = []
    for c in range(N_CORES):
        sl = slice(128 * c, 128 * c + 128)
        m = {
            "xt": xt,
            "wv": pack_w(np.asarray(Wv)[sl], bf16),
            "wo": np.ascontiguousarray(
                np.asarray(Wo, dtype=np.float32)[:, sl].T).astype(bf16),
            "bv": np.asarray(bv, dtype=np.float32)[sl].reshape(128, 1),
        }
        if QK_DR:
            m["xt8"] = xt8
            m["wq"] = pack_w(np.asarray(Wq)[sl], f8, QK_W_SCALE)
            m["wk"] = pack_w(np.asarray(Wk)[sl], f8, QK_W_SCALE)
            m["bq"] = (np.asarray(bq, dtype=np.float32)[sl] *
                       QK_W_SCALE).reshape(128, 1)
            m["bk"] = (np.asarray(bk, dtype=np.float32)[sl] *
                       QK_W_SCALE).reshape(128, 1)
        else:
            m["wq"] = pack_w(np.asarray(Wq)[sl], bf16)
            m["wk"] = pack_w(np.asarray(Wk)[sl], bf16)
            m["bq"] = np.asarray(bq, dtype=np.float32)[sl].reshape(128, 1)
            m["bk"] = np.asarray(bk, dtype=np.float32)[sl].reshape(128, 1)
        in_maps.append(m)
    return in_maps


def kernel(x, Wq, bq, Wk, bk, Wv, bv, Wo, bo):
    nc = _get_compiled()
    in_maps = _prep_inputs(x, Wq, bq, Wk, bk, Wv, bv, Wo, bo)
    res = run_bass_kernel_spmd(nc, in_maps, core_ids=list(range(N_CORES)))
    acc = np.zeros((NT, D), dtype=np.float32)
    for c in range(N_CORES):
        acc += np.asarray(res.results[c]["out16"]).astype(np.float32)
    acc += np.asarray(bo, dtype=np.float32)[None, :]
    return acc.reshape(B, S, D)


# revision 39
# speedup vs baseline: 1.0852x; 1.0238x over previous
"""MultiHeadAttention Trainium2 kernel, 8-way tensor-parallel by head.

Problem: B=4, S=2048, D=1024, 16 heads, d_k=64 (nn_MultiHeadAttention_67585605370071).

Sharding: each core owns 2 heads (128 of the 1024 hidden dims):
  Wq/Wk/Wv column-sharded by head, Wo row-sharded; the 8 partial outputs
  are summed on the host (the row-shard reduction) and bo is added there.

Dataflow per core (matmuls bf16 / fp8-DoubleRow, f32 PSUM accumulation):
  - host passes x pre-transposed (xT bf16 [1024, 8192]); Q/K/V projections
    run bf16 (QK_DR=True switches Q/K to fp8 DoubleRow for ~4us, at the
    cost of rel-err 1.5e-2 -> 2.0e-2)
  - V rotated to token-major layout via SBUF->SBUF DMA transposes, then
    quantized fp8 into the augmented layout [V_A | 1 | pad | V_B | 1 | pad]
    by the GpSimd engine; the ones column makes the AV matmul emit softmax
    denominators as row 64 of its PSUM tile
  - scoresT = Kt.T @ Qt per [128,512] half-tile; exp is split between the
    ScalarE ACT (table exp) and a custom DVE microcode op EXP16_POLY_ANT
    ((1 + ks + (ks)^2/2)^16, 8/8 uop stages) so neither engine saturates;
    both emit fp8
  - AV via fp8 DoubleRow matmuls (2 k-tiles per pass), trailing the
    scores/exp stream by TRAIL chunks so the PE never stalls on exp latency
  - normalize via DVE reciprocal + GpSimd partition_broadcast + DVE
    tensor_mul (no PE broadcast matmul)
  - out-proj per 128-token tile; PSUM->SBUF copies alternate between an ACT
    Copy and a DVE tensor_copy; projection bias-adds run as ACT
    Identity+bias (exp/identity/copy share one ACT table - no reloads)
  - emission is a continuous chunk stream: unit u's trailing AVs and
    normalize ride in unit u+1's first chunks, projection closures drain
    with ceiling pacing (fully inside their unit - the next unit's scores
    and trailing AVs depend on kt/vaug), and out-proj matmul/copy pairs
    drain through paced producer/consumer queues that keep the single
    po PSUM bank's write-after-read order identical to emission order
PSUM budget (8 banks): 4x score [128,512] + 2x AV accumulators [65,512]
  + 1 projection bank + 1 out-proj bank.
"""
import numpy as np
import ml_dtypes

import concourse.bass as bass
import concourse.bacc as bacc
import concourse.mybir as mybir
import concourse.tile as tile
from concourse.bass_utils import run_bass_kernel_spmd

import concourse.dve_ops as dve_ops_mod
from concourse.dve_spec import C0, C1, One, Spec, Src0, lower as dve_lower, sq
from concourse.dve_uop import DveOpSpec

BF16 = mybir.dt.bfloat16
F32 = mybir.dt.float32
F8 = mybir.dt.float8e4
bf16 = ml_dtypes.bfloat16
f8 = ml_dtypes.float8_e4m3

B, S, D = 4, 2048, 1024
NT = B * S            # 8192 tokens
N_CORES = 8
KT_PER_B = S // 128   # 16 k-tiles per batch
QB_PER_B = S // 512   # 4 query blocks per batch
TB_PER_B = S // 512   # 4 token blocks per batch (projection)
NCH = KT_PER_B // 2   # 8 score/exp chunks per attention unit

# fp8-e4m3 attention-weights path: expT and V quantized to fp8 so the AV
# matmul can use DoubleRow (2x PE throughput).
VG = 144              # vaug per-k-tile column group
HOFF = 72             # head B column offset inside a group

# Q/K projections in fp8 DoubleRow; weights scaled by 32 on the host, the
# 32*32 factor folded into the exp scale.
QK_DR = False
QK_W_SCALE = 32.0
EXP_SCALE = 0.125 / (QK_W_SCALE * QK_W_SCALE if QK_DR else 1.0)

# exp tiles handed to the DVE custom op instead of ScalarE, per unit
DVE_EXP = {(c, 1) for c in range(7)}
DVE_EXP_LAST = DVE_EXP  # last-batch override (no proj work to hide latency)
VBIAS_DVE = True
SPLIT_EXP = True     # exp per [128,512] half (4 sp bufs) vs full [128,1024]
SPLIT_OUTPROJ = False  # out-proj mm/copy as separate bg closures
NDRAIN = 3           # bg closures drained per chunk
SP_BUFS = 4          # score-PSUM buffers ([128,512] each)
TRAIL_G = 5          # chunks the AV stream trails scores/exp by
MC_POPS = 2          # out-proj producer/consumer pops per chunk
SH_BUFS = 2          # shared pp/po PSUM buffers

_COMPILED = None

# ---------------------------------------------------------------------------
# Custom DVE exp op: out = exp(in * scale) via (1 + ks + (ks)^2/2)^16,
# k = scale/16 — quadratic (4 ALU stages) + 4 squarings = 8/8 v3 stages.
# Rel err vs true exp: rms ~1e-4; after fp8 output quantization 99.96% of
# outputs are bit-identical to fp8(exp(x)).
EXP16_NAME = "EXP16_POLY_ANT"


def _exp16_consts(scale):
    k = scale / 16.0
    return (k * k / 2.0, k)


def _exp16_ref(in0, in1, s0, s1, imm2):
    b = ((in0.astype(np.float32) * s0 + s1) * in0 + 1.0).astype(np.float32)
    for _ in range(4):
        b = (b * b).astype(np.float32)
    return b


def _register_exp16():
    for op in dve_ops_mod.OPS:
        if op.name == EXP16_NAME:
            return op
    body = sq(sq(sq(sq((Src0 * C0 + C1) * Src0 + One))))
    spec = Spec(body=body, reference=_exp16_ref)
    row = max(dve_ops_mod._SUB_OPCODE_FOR_NAME.values()) + 1
    assert row < 0x20, "custom-DVE opcode rows exhausted"
    dve_ops_mod._SUB_OPCODE_FOR_NAME[EXP16_NAME] = row
    shas = {}
    for ver in ("v3", "v4"):
        s = DveOpSpec(name=EXP16_NAME, opcode=row,
                      uops=dve_lower(spec, ver=ver), rd1_en=False)
        shas[ver] = s.sha(ver)
    op = dve_ops_mod.DveOp(EXP16_NAME, spec, False, shas)
    dve_ops_mod.OPS.append(op)
    dve_ops_mod.CUSTOM_DVE_SPECS[EXP16_NAME] = spec
    return op


# ---------------------------------------------------------------------------
def _build():
    exp_op = _register_exp16()
    exp_s0, exp_s1 = _exp16_consts(EXP_SCALE)
    Ident = mybir.ActivationFunctionType.Identity
    CopyF = mybir.ActivationFunctionType.Copy

    nc = bacc.Bacc("TRN2", target_bir_lowering=False, debug=False,
                   num_devices=N_CORES)

    xt_p = nc.declare_dram_parameter("xt", [D, NT], BF16, isOutput=False)
    wv_p = nc.declare_dram_parameter("wv", [128, D], BF16, isOutput=False)
    wo_p = nc.declare_dram_parameter("wo", [128, D], BF16, isOutput=False)
    bq_p = nc.declare_dram_parameter("bq", [128, 1], F32, isOutput=False)
    bk_p = nc.declare_dram_parameter("bk", [128, 1], F32, isOutput=False)
    bv_p = nc.declare_dram_parameter("bv", [128, 1], F32, isOutput=False)
    if QK_DR:
        xt8_p = nc.declare_dram_parameter("xt8", [D, NT], F8, isOutput=False)
        wq_p = nc.declare_dram_parameter("wq", [128, D], F8, isOutput=False)
        wk_p = nc.declare_dram_parameter("wk", [128, D], F8, isOutput=False)
    else:
        wq_p = nc.declare_dram_parameter("wq", [128, D], BF16, isOutput=False)
        wk_p = nc.declare_dram_parameter("wk", [128, D], BF16, isOutput=False)
    out16_p = nc.declare_dram_parameter("out16", [NT, D], BF16, isOutput=True)

    WDT = F8 if QK_DR else BF16

    with tile.TileContext(nc) as tc:
        with (
            tc.tile_pool(name="consts", bufs=1) as consts,
            tc.tile_pool(name="xts", bufs=3) as xts_pool,
            tc.tile_pool(name="qkv", bufs=2) as qkv_pool,
            tc.tile_pool(name="vtrp", bufs=4) as vtr_pool,
            tc.tile_pool(name="expp", bufs=4) as exp_pool,
            tc.tile_pool(name="attnp", bufs=3) as attn_pool,
            tc.tile_pool(name="small", bufs=3) as small,
            tc.tile_pool(name="outp", bufs=3) as out_pool,
            tc.tile_pool(name="ps_sp", bufs=2, space=bass.MemorySpace.PSUM) as ps_sp,
            tc.tile_pool(name="ps_av", bufs=2, space=bass.MemorySpace.PSUM) as ps_av,
            tc.tile_pool(name="ps_pp", bufs=1, space=bass.MemorySpace.PSUM) as ps_pp,
            tc.tile_pool(name="ps_po", bufs=1, space=bass.MemorySpace.PSUM) as ps_po,
        ):
            wq = consts.tile([128, D], WDT, name="wq")
            wk = consts.tile([128, D], WDT, name="wk")
            wv = consts.tile([128, D], BF16, name="wv")
            wo = consts.tile([128, D], BF16, name="wo")
            bq = consts.tile([128, 1], F32, name="bq")
            bk = consts.tile([128, 1], F32, name="bk")
            bv = consts.tile([128, 1], F32, name="bv")
            nc.sync.dma_start(wq[:], wq_p[:])
            nc.sync.dma_start(wk[:], wk_p[:])
            nc.sync.dma_start(wv[:], wv_p[:])
            nc.sync.dma_start(wo[:], wo_p[:])
            nc.sync.dma_start(bq[:], bq_p[:])
            nc.sync.dma_start(bk[:], bk_p[:])
            nc.sync.dma_start(bv[:], bv_p[:])

            # per-batch, per-block persistent tiles, allocated lazily
            qt = {}          # qt[b][tb] -> [128, 512]
            kt = {}          # kt[b][tb] -> [128, 512] (4 k-tiles each)
            vts = {}         # vts[b][tb] -> [128, 512]
            vaug = {}        # vaug[b][tb] -> [128, 4*VG] fp8
            xts_t = {}       # prefetched xT stacks (bf16, fp8)

            def _alloc_batch(b):
                qt[b] = [qkv_pool.tile([128, 512], BF16, tag=f"qt{t}",
                                       name=f"qt{b}_{t}") for t in range(4)]
                kt[b] = [qkv_pool.tile([128, 512], BF16, tag=f"kt{t}",
                                       name=f"kt{b}_{t}") for t in range(4)]
                vts[b] = [qkv_pool.tile([128, 512], BF16, tag=f"vts{t}",
                                        name=f"vts{b}_{t}") for t in range(4)]
                vaug[b] = []
                for t in range(4):
                    va = qkv_pool.tile([128, 4 * VG], F8, tag=f"vaug{t}",
                                       name=f"vaug{b}_{t}")
                    va3 = va.rearrange("p (k c) -> p k c", c=VG)
                    nc.gpsimd.memset(va3[:, :, 64:65], 1.0)
                    nc.gpsimd.memset(va3[:, :, HOFF + 64:HOFF + 65], 1.0)
                    vaug[b].append(va)

            def emit_proj_dma(b, tb):
                """Prefetch the 512-token xT stack(s) for block (b, tb)."""
                if b not in qt:
                    _alloc_batch(b)
                tok0 = b * S + tb * 512
                xts = xts_pool.tile([128, 8 * 512], BF16, tag="xts",
                                    name=f"xts{b}_{tb}")
                src3 = xt_p.rearrange("(kd p) t -> p kd t", p=128)
                dst3 = xts.rearrange("p (kd t) -> p kd t", t=512)
                nc.sync.dma_start(dst3[:, :, :], src3[:, :, tok0:tok0 + 512])
                if QK_DR:
                    xts8 = xts_pool.tile([128, 8 * 512], F8, tag="xts8",
                                         name=f"xts8{b}_{tb}")
                    src83 = xt8_p.rearrange("(kd p) t -> p kd t", p=128)
                    dst83 = xts8.rearrange("p (kd t) -> p kd t", t=512)
                    nc.sync.dma_start(dst83[:, :, :], src83[:, :, tok0:tok0 + 512])
                    xts_t[(b, tb)] = (xts, xts8)
                else:
                    xts_t[(b, tb)] = (xts, xts)

            def emit_proj_compute(b, tb):
                for clo in proj_closures(b, tb):
                    clo()

            def proj_closures(b, tb):
                """The projection block as a list of small closures so its PE
                work can be interleaved between attention score chunks.
                V runs first so its rotation DMAs are in flight long before
                the Pool quantize copies (emitted last) need them."""
                xts, xts8 = xts_t.pop((b, tb))

                # Build producer (PE matmul) and consumer (ACT bias / Pool
                # quantize) closures, then interleave so every consumer
                # drains a few slots after its producer — an in-order
                # engine never head-of-line blocks on a PE matmul.
                ppk = ps_pp.tile([128, 512], F32, tag="pp", name=f"ppk{b}_{tb}")
                ppq = ps_pp.tile([128, 512], F32, tag="pp", name=f"ppq{b}_{tb}")
                ppv = ps_pp.tile([128, 512], F32, tag="pp", name=f"ppv{b}_{tb}")

                def mk_qk(pp, w_sb):
                    if QK_DR:
                        w3 = w_sb.rearrange("p (k m) -> p k m", m=128)
                        x3 = xts8.rearrange("p (k t) -> p k t", t=512)

                        def mmdr(p0):
                            for p in (p0, p0 + 1):
                                nc.tensor.matmul(
                                    pp[:], w3[:, 2 * p:2 * p + 2, :],
                                    x3[:, 2 * p:2 * p + 2, :],
                                    start=(p == 0), stop=(p == 3),
                                    perf_mode=mybir.MatmulPerfMode.DoubleRow)
                        return [lambda p0=p0: mmdr(p0) for p0 in (0, 2)]

                    def mm2(kd0):
                        for kd in (kd0, kd0 + 1):
                            nc.tensor.matmul(
                                pp[:], w_sb[:, 128 * kd:128 * kd + 128],
                                xts[:, 512 * kd:512 * kd + 512],
                                start=(kd == 0), stop=(kd == 7))
                    return [lambda kd0=kd0: mm2(kd0) for kd0 in (0, 2, 4, 6)]

                def mm2v(kd0):
                    for kd in (kd0, kd0 + 1):
                        nc.tensor.matmul(
                            ppv[:], wv[:, 128 * kd:128 * kd + 128],
                            xts[:, 512 * kd:512 * kd + 512],
                            start=(kd == 0), stop=(kd == 7))

                vtrs = [vtr_pool.tile([128, 128], BF16, tag=f"vtr{ti}",
                                      name=f"vtr{b}_{tb}_{ti}")
                        for ti in range(4)]

                def rot_dma():
                    for ti in range(4):
                        nc.sync.dma_start_transpose(
                            vtrs[ti][:], vts[b][tb][:, 128 * ti:128 * ti + 128])

                va4 = vaug[b][tb].rearrange("p (k g e) -> p k g e", g=2, e=HOFF)

                def rot_q(ti):
                    vtr3 = vtrs[ti].rearrange("p (g e) -> p g e", e=64)
                    nc.gpsimd.tensor_copy(va4[:, ti, :, 0:64], vtr3[:, :, :])

                kbias = lambda: nc.scalar.activation(kt[b][tb][:], ppk[:],
                                                     Ident, bias=bk[:])
                qbias = lambda: nc.scalar.activation(qt[b][tb][:], ppq[:],
                                                     Ident, bias=bq[:])
                if VBIAS_DVE:
                    vbias = lambda: nc.vector.tensor_scalar_add(
                        vts[b][tb][:], ppv[:], bv[:])
                else:
                    vbias = lambda: nc.scalar.activation(
                        vts[b][tb][:], ppv[:], Ident, bias=bv[:])
                kmm = mk_qk(ppk, wk)
                qmm = mk_qk(ppq, wq)
                vmm = [lambda kd0=kd0: mm2v(kd0) for kd0 in (0, 2, 4, 6)]
                return (kmm + [kbias] + qmm + [qbias] + vmm
                        + [vbias, rot_dma]
                        + [lambda ti=ti: rot_q(ti) for ti in range(4)])

            def outproj_closures(b, qb, attn):
                """Interleaved [mm, mm, copy, mm, copy, ...] so each
                PSUM->SBUF copy drains ~2 slots after its matmul."""
                obs = [out_pool.tile([128, 1024], BF16, tag="ob",
                                     name=f"ob{b}_{qb}_{ti}")
                       for ti in range(4)]
                pos = {}

                def op_mm(ti, j):
                    po = ps_po.tile([128, 512], F32, tag="po",
                                    name=f"po{b}_{qb}_{ti}_{j}")
                    nc.tensor.matmul(po[:],
                                     attn[:, 128 * ti:128 * ti + 128],
                                     wo[:, 512 * j:512 * j + 512],
                                     start=True, stop=True)
                    pos[(ti, j)] = po

                def op_cp(ti, j, on_act):
                    po = pos.pop((ti, j))
                    ob = obs[ti]
                    if on_act:
                        nc.scalar.activation(ob[:, 512 * j:512 * j + 512],
                                             po[:], CopyF)
                    else:
                        nc.vector.tensor_copy(ob[:, 512 * j:512 * j + 512],
                                              po[:])
                    if j == 1:
                        row0 = b * S + 512 * qb + 128 * ti
                        nc.sync.dma_start(out16_p[row0:row0 + 128, :], ob[:])

                halves = [(ti, j) for ti in range(4) for j in range(2)]
                mms = [lambda ti=ti, j=j: op_mm(ti, j) for ti, j in halves]
                cps = [lambda ti=ti, j=j, on_act=((ti + j) % 2 == 0):
                       op_cp(ti, j, on_act) for ti, j in halves]
                return mms, cps

            TRAIL = TRAIL_G

            class Unit:
                """Per-unit attention state for the continuous chunk stream."""

                def __init__(self, b, qb):
                    self.b, self.qb = b, qb
                    self.av = None
                    self.ecs = {}

                def emit_scores_exp(self, c):
                    b, qb = self.b, self.qb
                    qsl = qt[b][qb]
                    tb_of_c = c // 2        # source projection block
                    cc = c % 2              # k-tile pair within block
                    for h in range(2):
                        ec = exp_pool.tile([128, 1024], F8, tag=f"expt{h}",
                                           bufs=TRAIL_G + 2,
                                           name=f"ec{b}_{qb}_{c}_{h}")
                        if SPLIT_EXP:
                            for j in range(2):
                                sp = ps_sp.tile([128, 512], F32, tag="sp",
                                                bufs=SP_BUFS,
                                                name=f"sp{b}_{qb}_{c}_{h}_{j}")
                                kt_loc = 2 * cc + j
                                nc.tensor.matmul(
                                    sp[:],
                                    kt[b][tb_of_c][64 * h:64 * h + 64,
                                                   128 * kt_loc:128 * kt_loc + 128],
                                    qsl[64 * h:64 * h + 64, :],
                                    start=True, stop=True)
                                ech = ec[:, 512 * j:512 * j + 512]
                                if (c, h) in (DVE_EXP_LAST if b == B - 1 else DVE_EXP):
                                    nc.vector._custom_dve(exp_op, out=ech,
                                                          in0=sp[:],
                                                          s0=exp_s0, s1=exp_s1)
                                else:
                                    nc.scalar.activation(
                                        ech, sp[:],
                                        mybir.ActivationFunctionType.Exp,
                                        scale=EXP_SCALE)
                        else:
                            sp = ps_sp.tile([128, 1024], F32, tag="sp", bufs=2,
                                            name=f"sp{b}_{qb}_{c}_{h}")
                            for j in range(2):
                                kt_loc = 2 * cc + j
                                nc.tensor.matmul(
                                    sp[:, 512 * j:512 * j + 512],
                                    kt[b][tb_of_c][64 * h:64 * h + 64,
                                                   128 * kt_loc:128 * kt_loc + 128],
                                    qsl[64 * h:64 * h + 64, :],
                                    start=True, stop=True)
                            if (c, h) in (DVE_EXP_LAST if b == B - 1 else DVE_EXP):
                                nc.vector._custom_dve(exp_op, out=ec[:],
                                                      in0=sp[:],
                                                      s0=exp_s0, s1=exp_s1)
                            else:
                                nc.scalar.activation(
                                    ec[:], sp[:],
                                    mybir.ActivationFunctionType.Exp,
                                    scale=EXP_SCALE)
                        self.ecs[(c, h)] = ec

                def emit_av(self, cp):
                    b, qb = self.b, self.qb
                    if self.av is None:
                        self.av = [ps_av.tile([65, 512], F32, tag="av",
                                              name=f"av{b}_{qb}_{h}")
                                   for h in range(2)]
                    tb_p = cp // 2
                    ccp = cp % 2
                    va3 = vaug[b][tb_p].rearrange("p (k c) -> p k c", c=VG)
                    for h in range(2):
                        ec3 = self.ecs.pop((cp, h)).rearrange(
                            "p (k q) -> p k q", q=512)
                        nc.tensor.matmul(
                            self.av[h][:],
                            va3[:, 2 * ccp:2 * ccp + 2, HOFF * h:HOFF * h + 65],
                            ec3[:, :, :],
                            start=(cp == 0), stop=(cp == NCH - 1),
                            perf_mode=mybir.MatmulPerfMode.DoubleRow)

                def emit_normalize(self):
                    b, qb = self.b, self.qb
                    attn = attn_pool.tile([128, 512], BF16, tag="attn",
                                          name=f"attn{b}_{qb}")
                    rrows, bcs = [], []
                    for h in range(2):
                        rrow = small.tile([1, 512], F32, tag=f"rrow{h}",
                                          name=f"rr{b}_{qb}_{h}")
                        nc.vector.reciprocal(rrow[:], self.av[h][64:65, :])
                        rrows.append(rrow)
                    for h in range(2):
                        bc_sb = small.tile([64, 512], F32, tag=f"bc_sb{h}",
                                           name=f"bs{b}_{qb}_{h}")
                        nc.gpsimd.partition_broadcast(bc_sb[:], rrows[h])
                        bcs.append(bc_sb)
                    for h in range(2):
                        nc.vector.tensor_mul(attn[64 * h:64 * h + 64, :],
                                             self.av[h][0:64, :], bcs[h])
                    return attn

            # Continuous chunk stream: unit u's chunks 0..7 emit scores+exp;
            # its AV matmuls trail by TRAIL chunks, spilling into unit u+1's
            # first TRAIL chunks; normalize for unit u is emitted right after
            # its last AV (chunk TRAIL of unit u+1), and its out-proj drains
            # later in that unit via the bg queue. Projection blocks for
            # batch b+1 interleave throughout. No engine ever runs dry at a
            # unit boundary.
            from collections import deque
            bgA = deque()        # projection closures (internally ordered)
            bgM = deque()        # out-proj matmuls (producers)
            bgC = deque()        # out-proj PSUM->SBUF copies (consumers)
            nM = nC = 0          # popped counts for producer/consumer pacing
            units = [(b, qb) for b in range(B) for qb in range(QB_PER_B)]

            def drain_mc():
                nonlocal nM, nC
                # po pool is single-buffered: the copy of matmul i must be
                # emitted before matmul i+1 (WAR on the recycled bank)
                if bgC and (nM - nC >= 1 or not bgM):
                    bgC.popleft()()
                    nC += 1
                elif bgM:
                    bgM.popleft()()
                    nM += 1

            for tb in range(TB_PER_B):
                emit_proj_dma(0, tb)
            emit_proj_compute(0, 0)
            bgA.extend(proj_closures(0, 1))
            bgA.extend(proj_closures(0, 2))
            bgA.extend(proj_closures(0, 3))

            prev = None          # unit with AV trail / normalize outstanding
            pend_out = None      # out-proj closures awaiting queue insertion
            for u, (b, qb) in enumerate(units):
                cur = Unit(b, qb)
                carry = 0
                if b + 1 < B:
                    emit_proj_dma(b + 1, qb)
                    newc = proj_closures(b + 1, qb)
                    bgA.extend(newc)
                    # the block added this unit may spill into the next unit:
                    # its kt/vaug are first read >=1 unit later
                    carry = len(newc)
                if pend_out is not None:
                    bgM.extend(pend_out[0])
                    bgC.extend(pend_out[1])
                    pend_out = None
                for c in range(NCH):
                    cur.emit_scores_exp(c)
                    if c >= TRAIL:
                        cur.emit_av(c - TRAIL)
                    elif prev is not None:
                        prev.emit_av(NCH - TRAIL + c)
                        if c == TRAIL - 1:
                            attn = prev.emit_normalize()
                            pend_out = outproj_closures(prev.b, prev.qb, attn)
                    # pace the projection queue so it fully drains within
                    # this unit: chunk c of NCH has (NCH - c) chunks left,
                    # and the next unit's trailing AVs / scores depend on
                    # this unit's projection outputs (kt, vaug)
                    na = max(2, -(-max(0, len(bgA) - carry) // (NCH - c)))
                    for _ in range(na):
                        if bgA:
                            bgA.popleft()()
                    for _ in range(MC_POPS):
                        drain_mc()
                prev = cur
            for c in range(TRAIL):
                prev.emit_av(NCH - TRAIL + c)
            attn = prev.emit_normalize()
            if pend_out is not None:
                bgM.extend(pend_out[0])
                bgC.extend(pend_out[1])
            mo, co = outproj_closures(prev.b, prev.qb, attn)
            bgM.extend(mo)
            bgC.extend(co)
            while bgA or bgM or bgC:
                if bgA:
                    bgA.popleft()()
                drain_mc()

    nc.compile()
    return nc


def _get_compiled():
    global _COMPILED
    if _COMPILED is None:
        _COMPILED = _build()
    return _COMPILED


def _prep_inputs(x, Wq, bq, Wk, bk, Wv, bv, Wo, bo):
    xf = np.asarray(x, dtype=np.float32).reshape(NT, D).T
    xt = np.ascontiguousarray(xf).astype(bf16)

    def pack_w(Wc, dtype, scale=1.0):
        # [128 out, 1024 in] -> k-tile packed [128, 1024]
        wt = np.asarray(Wc, dtype=np.float32).T * scale  # [1024 in, 128 out]
        return np.ascontiguousarray(
            wt.reshape(8, 128, 128).transpose(1, 0, 2).reshape(128, D)).astype(dtype)

    if QK_DR:
        xt8 = np.ascontiguousarray(xf).astype(f8)

    in_maps = []
    for c in range(N_CORES):
        sl = slice(128 * c, 128 * c + 128)
        m = {
            "xt": xt,
            "wv": pack_w(np.asarray(Wv)[sl], bf16),
            "wo": np.ascontiguousarray(
                np.asarray(Wo, dtype=np.float32)[:, sl].T).astype(bf16),
            "bv": np.asarray(bv, dtype=np.float32)[sl].reshape(128, 1),
        }
        if QK_DR:
            m["xt8"] = xt8
            m["wq"] = pack_w(np.asarray(Wq)[sl], f8, QK_W_SCALE)
            m["wk"] = pack_w(np.asarray(Wk)[sl], f8, QK_W_SCALE)
            m["bq"] = (np.asarray(bq, dtype=np.float32)[sl] *
                       QK_W_SCALE).reshape(128, 1)
            m["bk"] = (np.asarray(bk, dtype=np.float32)[sl] *
                       QK_W_SCALE).reshape(128, 1)
        else:
            m["wq"] = pack_w(np.asarray(Wq)[sl], bf16)
            m["wk"] = pack_w(np.asarray(Wk)[sl], bf16)
            m["bq"] = np.asarray(bq, dtype=np.float32)[sl].reshape(128, 1)
            m["bk"] = np.asarray(bk, dtype=np.float32)[sl].reshape(128, 1)
        in_maps.append(m)
    return in_maps


def kernel(x, Wq, bq, Wk, bk, Wv, bv, Wo, bo):
    nc = _get_compiled()
    in_maps = _prep_inputs(x, Wq, bq, Wk, bk, Wv, bv, Wo, bo)
    res = run_bass_kernel_spmd(nc, in_maps, core_ids=list(range(N_CORES)))
    acc = np.zeros((NT, D), dtype=np.float32)
    for c in range(N_CORES):
        acc += np.asarray(res.results[c]["out16"]).astype(np.float32)
    acc += np.asarray(bo, dtype=np.float32)[None, :]
    return acc.reshape(B, S, D)


# revision 45
# speedup vs baseline: 1.0886x; 1.0032x over previous
"""MultiHeadAttention Trainium2 kernel, 8-way tensor-parallel by head.

Problem: B=4, S=2048, D=1024, 16 heads, d_k=64 (nn_MultiHeadAttention_67585605370071).

Sharding: each core owns 2 heads (128 of the 1024 hidden dims):
  Wq/Wk/Wv column-sharded by head, Wo row-sharded; the 8 partial outputs
  are summed on the host (the row-shard reduction) and bo is added there.

Dataflow per core (matmuls bf16 / fp8-DoubleRow, f32 PSUM accumulation):
  - host passes x pre-transposed (xT bf16 [1024, 8192]); Q/K/V projections
    run bf16 (QK_DR=True switches Q/K to fp8 DoubleRow for ~4us, at the
    cost of rel-err 1.5e-2 -> 2.0e-2)
  - V rotated to token-major layout via SBUF->SBUF DMA transposes, then
    quantized fp8 into the augmented layout [V_A | 1 | pad | V_B | 1 | pad]
    by the GpSimd engine; the ones column makes the AV matmul emit softmax
    denominators as row 64 of its PSUM tile
  - scoresT = Kt.T @ Qt per [128,512] half-tile; exp is split between the
    ScalarE ACT (table exp) and a custom DVE microcode op EXP16_POLY_ANT
    ((1 + ks + (ks)^2/2)^16, 8/8 uop stages) so neither engine saturates;
    both emit fp8
  - AV via fp8 DoubleRow matmuls (2 k-tiles per pass), trailing the
    scores/exp stream by TRAIL chunks so the PE never stalls on exp latency
  - normalize via DVE reciprocal + GpSimd partition_broadcast + DVE
    tensor_mul (no PE broadcast matmul)
  - out-proj per 128-token tile; PSUM->SBUF copies alternate between an ACT
    Copy and a DVE tensor_copy; projection bias-adds run as ACT
    Identity+bias (exp/identity/copy share one ACT table - no reloads)
  - emission is a continuous chunk stream: unit u's trailing AVs and
    normalize ride in unit u+1's first chunks, projection closures drain
    with ceiling pacing (fully inside their unit - the next unit's scores
    and trailing AVs depend on kt/vaug), and out-proj matmul/copy pairs
    drain through paced producer/consumer queues that keep the single
    po PSUM bank's write-after-read order identical to emission order
PSUM budget (8 banks): 4x score [128,512] + 2x AV accumulators [65,512]
  + 1 projection bank + 1 out-proj bank.
"""
import numpy as np
import ml_dtypes

import concourse.bass as bass
import concourse.bacc as bacc
import concourse.mybir as mybir
import concourse.tile as tile
from concourse.bass_utils import run_bass_kernel_spmd

import concourse.dve_ops as dve_ops_mod
from concourse.dve_spec import C0, C1, One, Spec, Src0, lower as dve_lower, sq
from concourse.dve_uop import DveOpSpec

BF16 = mybir.dt.bfloat16
F32 = mybir.dt.float32
F8 = mybir.dt.float8e4
bf16 = ml_dtypes.bfloat16
f8 = ml_dtypes.float8_e4m3

B, S, D = 4, 2048, 1024
NT = B * S            # 8192 tokens
N_CORES = 8
KT_PER_B = S // 128   # 16 k-tiles per batch
QB_PER_B = S // 512   # 4 query blocks per batch
TB_PER_B = S // 512   # 4 token blocks per batch (projection)
NCH = KT_PER_B // 2   # 8 score/exp chunks per attention unit

# fp8-e4m3 attention-weights path: expT and V quantized to fp8 so the AV
# matmul can use DoubleRow (2x PE throughput).
VG = 144              # vaug per-k-tile column group
HOFF = 72             # head B column offset inside a group

# Q/K projections in fp8 DoubleRow; weights scaled by 32 on the host, the
# 32*32 factor folded into the exp scale.
QK_DR = False
QK_W_SCALE = 32.0
EXP_SCALE = 0.125 / (QK_W_SCALE * QK_W_SCALE if QK_DR else 1.0)

# exp tiles handed to the DVE custom op instead of ScalarE, per unit
DVE_EXP = {(c, 1) for c in range(7)}
DVE_EXP_LAST = DVE_EXP  # last-batch override (no proj work to hide latency)
VBIAS_DVE = True
SPLIT_EXP = True     # exp per [128,512] half (4 sp bufs) vs full [128,1024]
SPLIT_OUTPROJ = False  # out-proj mm/copy as separate bg closures
NDRAIN = 3           # bg closures drained per chunk
SP_BUFS = 4          # score-PSUM buffers ([128,512] each)
TRAIL_G = 5          # chunks the AV stream trails scores/exp by
MC_POPS = 2          # out-proj producer/consumer pops per chunk
SH_BUFS = 2          # shared pp/po PSUM buffers

_COMPILED = None

# ---------------------------------------------------------------------------
# Custom DVE exp op: out = exp(in * scale) via (1 + ks + (ks)^2/2)^16,
# k = scale/16 — quadratic (4 ALU stages) + 4 squarings = 8/8 v3 stages.
# Rel err vs true exp: rms ~1e-4; after fp8 output quantization 99.96% of
# outputs are bit-identical to fp8(exp(x)).
EXP16_NAME = "EXP16_POLY_ANT"


def _exp16_consts(scale):
    k = scale / 16.0
    return (k * k / 2.0, k)


def _exp16_ref(in0, in1, s0, s1, imm2):
    b = ((in0.astype(np.float32) * s0 + s1) * in0 + 1.0).astype(np.float32)
    for _ in range(4):
        b = (b * b).astype(np.float32)
    return b


def _register_exp16():
    for op in dve_ops_mod.OPS:
        if op.name == EXP16_NAME:
            return op
    body = sq(sq(sq(sq((Src0 * C0 + C1) * Src0 + One))))
    spec = Spec(body=body, reference=_exp16_ref)
    row = max(dve_ops_mod._SUB_OPCODE_FOR_NAME.values()) + 1
    assert row < 0x20, "custom-DVE opcode rows exhausted"
    dve_ops_mod._SUB_OPCODE_FOR_NAME[EXP16_NAME] = row
    shas = {}
    for ver in ("v3", "v4"):
        s = DveOpSpec(name=EXP16_NAME, opcode=row,
                      uops=dve_lower(spec, ver=ver), rd1_en=False)
        shas[ver] = s.sha(ver)
    op = dve_ops_mod.DveOp(EXP16_NAME, spec, False, shas)
    dve_ops_mod.OPS.append(op)
    dve_ops_mod.CUSTOM_DVE_SPECS[EXP16_NAME] = spec
    return op


# ---------------------------------------------------------------------------
def _build():
    exp_op = _register_exp16()
    exp_s0, exp_s1 = _exp16_consts(EXP_SCALE)
    Ident = mybir.ActivationFunctionType.Identity
    CopyF = mybir.ActivationFunctionType.Copy

    nc = bacc.Bacc("TRN2", target_bir_lowering=False, debug=False,
                   num_devices=N_CORES)

    xt_p = nc.declare_dram_parameter("xt", [D, NT], BF16, isOutput=False)
    wv_p = nc.declare_dram_parameter("wv", [128, D], BF16, isOutput=False)
    wo_p = nc.declare_dram_parameter("wo", [128, D], BF16, isOutput=False)
    bq_p = nc.declare_dram_parameter("bq", [128, 1], F32, isOutput=False)
    bk_p = nc.declare_dram_parameter("bk", [128, 1], F32, isOutput=False)
    bv_p = nc.declare_dram_parameter("bv", [128, 1], F32, isOutput=False)
    if QK_DR:
        xt8_p = nc.declare_dram_parameter("xt8", [D, NT], F8, isOutput=False)
        wq_p = nc.declare_dram_parameter("wq", [128, D], F8, isOutput=False)
        wk_p = nc.declare_dram_parameter("wk", [128, D], F8, isOutput=False)
    else:
        wq_p = nc.declare_dram_parameter("wq", [128, D], BF16, isOutput=False)
        wk_p = nc.declare_dram_parameter("wk", [128, D], BF16, isOutput=False)
    out16_p = nc.declare_dram_parameter("out16", [NT, D], BF16, isOutput=True)

    WDT = F8 if QK_DR else BF16

    with tile.TileContext(nc) as tc:
        with (
            tc.tile_pool(name="consts", bufs=1) as consts,
            tc.tile_pool(name="xts", bufs=4) as xts_pool,
            tc.tile_pool(name="qkv", bufs=3) as qkv_pool,
            tc.tile_pool(name="vtrp", bufs=6) as vtr_pool,
            tc.tile_pool(name="expp", bufs=4) as exp_pool,
            tc.tile_pool(name="attnp", bufs=4) as attn_pool,
            tc.tile_pool(name="small", bufs=4) as small,
            tc.tile_pool(name="outp", bufs=4) as out_pool,
            tc.tile_pool(name="ps_sp", bufs=2, space=bass.MemorySpace.PSUM) as ps_sp,
            tc.tile_pool(name="ps_av", bufs=2, space=bass.MemorySpace.PSUM) as ps_av,
            tc.tile_pool(name="ps_pp", bufs=1, space=bass.MemorySpace.PSUM) as ps_pp,
            tc.tile_pool(name="ps_po", bufs=1, space=bass.MemorySpace.PSUM) as ps_po,
        ):
            wq = consts.tile([128, D], WDT, name="wq")
            wk = consts.tile([128, D], WDT, name="wk")
            wv = consts.tile([128, D], BF16, name="wv")
            wo = consts.tile([128, D], BF16, name="wo")
            bq = consts.tile([128, 1], F32, name="bq")
            bk = consts.tile([128, 1], F32, name="bk")
            bv = consts.tile([128, 1], F32, name="bv")
            nc.sync.dma_start(wq[:], wq_p[:])
            nc.sync.dma_start(wk[:], wk_p[:])
            nc.sync.dma_start(wv[:], wv_p[:])
            nc.sync.dma_start(wo[:], wo_p[:])
            nc.sync.dma_start(bq[:], bq_p[:])
            nc.sync.dma_start(bk[:], bk_p[:])
            nc.sync.dma_start(bv[:], bv_p[:])

            # per-batch, per-block persistent tiles, allocated lazily
            qt = {}          # qt[b][tb] -> [128, 512]
            kt = {}          # kt[b][tb] -> [128, 512] (4 k-tiles each)
            vts = {}         # vts[b][tb] -> [128, 512]
            vaug = {}        # vaug[b][tb] -> [128, 4*VG] fp8
            xts_t = {}       # prefetched xT stacks (bf16, fp8)

            def _alloc_batch(b):
                qt[b] = [qkv_pool.tile([128, 512], BF16, tag=f"qt{t}",
                                       name=f"qt{b}_{t}") for t in range(4)]
                kt[b] = [qkv_pool.tile([128, 512], BF16, tag=f"kt{t}",
                                       name=f"kt{b}_{t}") for t in range(4)]
                vts[b] = [qkv_pool.tile([128, 512], BF16, tag=f"vts{t}",
                                        name=f"vts{b}_{t}") for t in range(4)]
                vaug[b] = []
                for t in range(4):
                    va = qkv_pool.tile([128, 4 * VG], F8, tag=f"vaug{t}",
                                       name=f"vaug{b}_{t}")
                    va3 = va.rearrange("p (k c) -> p k c", c=VG)
                    nc.gpsimd.memset(va3[:, :, 64:65], 1.0)
                    nc.gpsimd.memset(va3[:, :, HOFF + 64:HOFF + 65], 1.0)
                    vaug[b].append(va)

            def emit_proj_dma(b, tb):
                """Prefetch the 512-token xT stack(s) for block (b, tb)."""
                if b not in qt:
                    _alloc_batch(b)
                tok0 = b * S + tb * 512
                xts = xts_pool.tile([128, 8 * 512], BF16, tag="xts",
                                    name=f"xts{b}_{tb}")
                src3 = xt_p.rearrange("(kd p) t -> p kd t", p=128)
                dst3 = xts.rearrange("p (kd t) -> p kd t", t=512)
                nc.sync.dma_start(dst3[:, :, :], src3[:, :, tok0:tok0 + 512])
                if QK_DR:
                    xts8 = xts_pool.tile([128, 8 * 512], F8, tag="xts8",
                                         name=f"xts8{b}_{tb}")
                    src83 = xt8_p.rearrange("(kd p) t -> p kd t", p=128)
                    dst83 = xts8.rearrange("p (kd t) -> p kd t", t=512)
                    nc.sync.dma_start(dst83[:, :, :], src83[:, :, tok0:tok0 + 512])
                    xts_t[(b, tb)] = (xts, xts8)
                else:
                    xts_t[(b, tb)] = (xts, xts)

            def emit_proj_compute(b, tb):
                for clo in proj_closures(b, tb):
                    clo()

            def proj_closures(b, tb):
                """The projection block as a list of small closures so its PE
                work can be interleaved between attention score chunks.
                V runs first so its rotation DMAs are in flight long before
                the Pool quantize copies (emitted last) need them."""
                xts, xts8 = xts_t.pop((b, tb))

                # Build producer (PE matmul) and consumer (ACT bias / Pool
                # quantize) closures, then interleave so every consumer
                # drains a few slots after its producer — an in-order
                # engine never head-of-line blocks on a PE matmul.
                ppk = ps_pp.tile([128, 512], F32, tag="pp", name=f"ppk{b}_{tb}")
                ppq = ps_pp.tile([128, 512], F32, tag="pp", name=f"ppq{b}_{tb}")
                ppv = ps_pp.tile([128, 512], F32, tag="pp", name=f"ppv{b}_{tb}")

                def mk_qk(pp, w_sb):
                    if QK_DR:
                        w3 = w_sb.rearrange("p (k m) -> p k m", m=128)
                        x3 = xts8.rearrange("p (k t) -> p k t", t=512)

                        def mmdr(p0):
                            for p in (p0, p0 + 1):
                                nc.tensor.matmul(
                                    pp[:], w3[:, 2 * p:2 * p + 2, :],
                                    x3[:, 2 * p:2 * p + 2, :],
                                    start=(p == 0), stop=(p == 3),
                                    perf_mode=mybir.MatmulPerfMode.DoubleRow)
                        return [lambda p0=p0: mmdr(p0) for p0 in (0, 2)]

                    def mm2(kd0):
                        for kd in (kd0, kd0 + 1):
                            nc.tensor.matmul(
                                pp[:], w_sb[:, 128 * kd:128 * kd + 128],
                                xts[:, 512 * kd:512 * kd + 512],
                                start=(kd == 0), stop=(kd == 7))
                    return [lambda kd0=kd0: mm2(kd0) for kd0 in (0, 2, 4, 6)]

                def mm2v(kd0):
                    for kd in (kd0, kd0 + 1):
                        nc.tensor.matmul(
                            ppv[:], wv[:, 128 * kd:128 * kd + 128],
                            xts[:, 512 * kd:512 * kd + 512],
                            start=(kd == 0), stop=(kd == 7))

                vtrs = [vtr_pool.tile([128, 128], BF16, tag=f"vtr{ti}",
                                      name=f"vtr{b}_{tb}_{ti}")
                        for ti in range(4)]

                def rot_dma():
                    for ti in range(4):
                        nc.sync.dma_start_transpose(
                            vtrs[ti][:], vts[b][tb][:, 128 * ti:128 * ti + 128])

                va4 = vaug[b][tb].rearrange("p (k g e) -> p k g e", g=2, e=HOFF)

                def rot_q(ti):
                    vtr3 = vtrs[ti].rearrange("p (g e) -> p g e", e=64)
                    nc.gpsimd.tensor_copy(va4[:, ti, :, 0:64], vtr3[:, :, :])

                kbias = lambda: nc.scalar.activation(kt[b][tb][:], ppk[:],
                                                     Ident, bias=bk[:])
                qbias = lambda: nc.scalar.activation(qt[b][tb][:], ppq[:],
                                                     Ident, bias=bq[:])
                if VBIAS_DVE:
                    vbias = lambda: nc.vector.tensor_scalar_add(
                        vts[b][tb][:], ppv[:], bv[:])
                else:
                    vbias = lambda: nc.scalar.activation(
                        vts[b][tb][:], ppv[:], Ident, bias=bv[:])
                kmm = mk_qk(ppk, wk)
                qmm = mk_qk(ppq, wq)
                vmm = [lambda kd0=kd0: mm2v(kd0) for kd0 in (0, 2, 4, 6)]
                return (kmm + [kbias] + qmm + [qbias] + vmm
                        + [vbias, rot_dma]
                        + [lambda ti=ti: rot_q(ti) for ti in range(4)])

            def outproj_closures(b, qb, attn):
                """Interleaved [mm, mm, copy, mm, copy, ...] so each
                PSUM->SBUF copy drains ~2 slots after its matmul."""
                obs = [out_pool.tile([128, 1024], BF16, tag="ob",
                                     name=f"ob{b}_{qb}_{ti}")
                       for ti in range(4)]
                pos = {}

                def op_mm(ti, j):
                    po = ps_po.tile([128, 512], F32, tag="po",
                                    name=f"po{b}_{qb}_{ti}_{j}")
                    nc.tensor.matmul(po[:],
                                     attn[:, 128 * ti:128 * ti + 128],
                                     wo[:, 512 * j:512 * j + 512],
                                     start=True, stop=True)
                    pos[(ti, j)] = po

                def op_cp(ti, j, on_act):
                    po = pos.pop((ti, j))
                    ob = obs[ti]
                    if on_act:
                        nc.scalar.activation(ob[:, 512 * j:512 * j + 512],
                                             po[:], CopyF)
                    else:
                        nc.vector.tensor_copy(ob[:, 512 * j:512 * j + 512],
                                              po[:])
                    if j == 1:
                        row0 = b * S + 512 * qb + 128 * ti
                        nc.sync.dma_start(out16_p[row0:row0 + 128, :], ob[:])

                halves = [(ti, j) for ti in range(4) for j in range(2)]
                mms = [lambda ti=ti, j=j: op_mm(ti, j) for ti, j in halves]
                cps = [lambda ti=ti, j=j, on_act=((ti + j) % 2 == 0):
                       op_cp(ti, j, on_act) for ti, j in halves]
                return mms, cps

            TRAIL = TRAIL_G

            class Unit:
                """Per-unit attention state for the continuous chunk stream."""

                def __init__(self, b, qb):
                    self.b, self.qb = b, qb
                    self.av = None
                    self.ecs = {}

                def emit_scores_exp(self, c):
                    b, qb = self.b, self.qb
                    qsl = qt[b][qb]
                    tb_of_c = c // 2        # source projection block
                    cc = c % 2              # k-tile pair within block
                    for h in range(2):
                        ec = exp_pool.tile([128, 1024], F8, tag=f"expt{h}",
                                           bufs=TRAIL_G + 3,
                                           name=f"ec{b}_{qb}_{c}_{h}")
                        if SPLIT_EXP:
                            for j in range(2):
                                sp = ps_sp.tile([128, 512], F32, tag="sp",
                                                bufs=SP_BUFS,
                                                name=f"sp{b}_{qb}_{c}_{h}_{j}")
                                kt_loc = 2 * cc + j
                                nc.tensor.matmul(
                                    sp[:],
                                    kt[b][tb_of_c][64 * h:64 * h + 64,
                                                   128 * kt_loc:128 * kt_loc + 128],
                                    qsl[64 * h:64 * h + 64, :],
                                    start=True, stop=True)
                                ech = ec[:, 512 * j:512 * j + 512]
                                if (c, h) in (DVE_EXP_LAST if b == B - 1 else DVE_EXP):
                                    nc.vector._custom_dve(exp_op, out=ech,
                                                          in0=sp[:],
                                                          s0=exp_s0, s1=exp_s1)
                                else:
                                    nc.scalar.activation(
                                        ech, sp[:],
                                        mybir.ActivationFunctionType.Exp,
                                        scale=EXP_SCALE)
                        else:
                            sp = ps_sp.tile([128, 1024], F32, tag="sp", bufs=2,
                                            name=f"sp{b}_{qb}_{c}_{h}")
                            for j in range(2):
                                kt_loc = 2 * cc + j
                                nc.tensor.matmul(
                                    sp[:, 512 * j:512 * j + 512],
                                    kt[b][tb_of_c][64 * h:64 * h + 64,
                                                   128 * kt_loc:128 * kt_loc + 128],
                                    qsl[64 * h:64 * h + 64, :],
                                    start=True, stop=True)
                            if (c, h) in (DVE_EXP_LAST if b == B - 1 else DVE_EXP):
                                nc.vector._custom_dve(exp_op, out=ec[:],
                                                      in0=sp[:],
                                                      s0=exp_s0, s1=exp_s1)
                            else:
                                nc.scalar.activation(
                                    ec[:], sp[:],
                                    mybir.ActivationFunctionType.Exp,
                                    scale=EXP_SCALE)
                        self.ecs[(c, h)] = ec

                def emit_av(self, cp):
                    b, qb = self.b, self.qb
                    if self.av is None:
                        self.av = [ps_av.tile([65, 512], F32, tag="av",
                                              name=f"av{b}_{qb}_{h}")
                                   for h in range(2)]
                    tb_p = cp // 2
                    ccp = cp % 2
                    va3 = vaug[b][tb_p].rearrange("p (k c) -> p k c", c=VG)
                    for h in range(2):
                        ec3 = self.ecs.pop((cp, h)).rearrange(
                            "p (k q) -> p k q", q=512)
                        nc.tensor.matmul(
                            self.av[h][:],
                            va3[:, 2 * ccp:2 * ccp + 2, HOFF * h:HOFF * h + 65],
                            ec3[:, :, :],
                            start=(cp == 0), stop=(cp == NCH - 1),
                            perf_mode=mybir.MatmulPerfMode.DoubleRow)

                def emit_normalize(self):
                    b, qb = self.b, self.qb
                    attn = attn_pool.tile([128, 512], BF16, tag="attn",
                                          name=f"attn{b}_{qb}")
                    rrows, bcs = [], []
                    for h in range(2):
                        rrow = small.tile([1, 512], F32, tag=f"rrow{h}",
                                          name=f"rr{b}_{qb}_{h}")
                        nc.vector.reciprocal(rrow[:], self.av[h][64:65, :])
                        rrows.append(rrow)
                    for h in range(2):
                        bc_sb = small.tile([64, 512], F32, tag=f"bc_sb{h}",
                                           name=f"bs{b}_{qb}_{h}")
                        nc.gpsimd.partition_broadcast(bc_sb[:], rrows[h])
                        bcs.append(bc_sb)
                    for h in range(2):
                        nc.vector.tensor_mul(attn[64 * h:64 * h + 64, :],
                                             self.av[h][0:64, :], bcs[h])
                    return attn

            # Continuous chunk stream: unit u's chunks 0..7 emit scores+exp;
            # its AV matmuls trail by TRAIL chunks, spilling into unit u+1's
            # first TRAIL chunks; normalize for unit u is emitted right after
            # its last AV (chunk TRAIL of unit u+1), and its out-proj drains
            # later in that unit via the bg queue. Projection blocks for
            # batch b+1 interleave throughout. No engine ever runs dry at a
            # unit boundary.
            from collections import deque
            bgA = deque()        # projection closures (internally ordered)
            bgM = deque()        # out-proj matmuls (producers)
            bgC = deque()        # out-proj PSUM->SBUF copies (consumers)
            nM = nC = 0          # popped counts for producer/consumer pacing
            units = [(b, qb) for b in range(B) for qb in range(QB_PER_B)]

            def drain_mc():
                nonlocal nM, nC
                # po pool is single-buffered: the copy of matmul i must be
                # emitted before matmul i+1 (WAR on the recycled bank)
                if bgC and (nM - nC >= 1 or not bgM):
                    bgC.popleft()()
                    nC += 1
                elif bgM:
                    bgM.popleft()()
                    nM += 1

            for tb in range(TB_PER_B):
                emit_proj_dma(0, tb)
            emit_proj_compute(0, 0)
            bgA.extend(proj_closures(0, 1))
            bgA.extend(proj_closures(0, 2))
            bgA.extend(proj_closures(0, 3))

            prev = None          # unit with AV trail / normalize outstanding
            pend_out = None      # out-proj closures awaiting queue insertion
            for u, (b, qb) in enumerate(units):
                cur = Unit(b, qb)
                carry = 0
                if b + 1 < B:
                    emit_proj_dma(b + 1, qb)
                    newc = proj_closures(b + 1, qb)
                    bgA.extend(newc)
                    # the block added this unit may spill into the next unit:
                    # its kt/vaug are first read >=1 unit later
                    carry = len(newc)
                if pend_out is not None:
                    bgM.extend(pend_out[0])
                    bgC.extend(pend_out[1])
                    pend_out = None
                for c in range(NCH):
                    cur.emit_scores_exp(c)
                    if c >= TRAIL:
                        cur.emit_av(c - TRAIL)
                    elif prev is not None:
                        prev.emit_av(NCH - TRAIL + c)
                        if c == TRAIL - 1:
                            attn = prev.emit_normalize()
                            pend_out = outproj_closures(prev.b, prev.qb, attn)
                    # pace the projection queue so it fully drains within
                    # this unit: chunk c of NCH has (NCH - c) chunks left,
                    # and the next unit's trailing AVs / scores depend on
                    # this unit's projection outputs (kt, vaug)
                    na = max(2, -(-max(0, len(bgA) - carry) // (NCH - c)))
                    for _ in range(na):
                        if bgA:
                            bgA.popleft()()
                    for _ in range(MC_POPS):
                        drain_mc()
                prev = cur
            for c in range(TRAIL):
                prev.emit_av(NCH - TRAIL + c)
            attn = prev.emit_normalize()
            if pend_out is not None:
                bgM.extend(pend_out[0])
                bgC.extend(pend_out[1])
            mo, co = outproj_closures(prev.b, prev.qb, attn)
            bgM.extend(mo)
            bgC.extend(co)
            while bgA or bgM or bgC:
                if bgA:
                    bgA.popleft()()
                drain_mc()

    nc.compile()
    return nc


def _get_compiled():
    global _COMPILED
    if _COMPILED is None:
        _COMPILED = _build()
    return _COMPILED


def _prep_inputs(x, Wq, bq, Wk, bk, Wv, bv, Wo, bo):
    xf = np.asarray(x, dtype=np.float32).reshape(NT, D).T
    xt = np.ascontiguousarray(xf).astype(bf16)

    def pack_w(Wc, dtype, scale=1.0):
        # [128 out, 1024 in] -> k-tile packed [128, 1024]
        wt = np.asarray(Wc, dtype=np.float32).T * scale  # [1024 in, 128 out]
        return np.ascontiguousarray(
            wt.reshape(8, 128, 128).transpose(1, 0, 2).reshape(128, D)).astype(dtype)

    if QK_DR:
        xt8 = np.ascontiguousarray(xf).astype(f8)

    in_maps = []
    for c in range(N_CORES):
        sl = slice(128 * c, 128 * c + 128)
        m = {
            "xt": xt,
            "wv": pack_w(np.asarray(Wv)[sl], bf16),
            "wo": np.ascontiguousarray(
                np.asarray(Wo, dtype=np.float32)[:, sl].T).astype(bf16),
            "bv": np.asarray(bv, dtype=np.float32)[sl].reshape(128, 1),
        }
        if QK_DR:
            m["xt8"] = xt8
            m["wq"] = pack_w(np.asarray(Wq)[sl], f8, QK_W_SCALE)
            m["wk"] = pack_w(np.asarray(Wk)[sl], f8, QK_W_SCALE)
            m["bq"] = (np.asarray(bq, dtype=np.float32)[sl] *
                       QK_W_SCALE).reshape(128, 1)
            m["bk"] = (np.asarray(bk, dtype=np.float32)[sl] *
                       QK_W_SCALE).reshape(128, 1)
        else:
            m["wq"] = pack_w(np.asarray(Wq)[sl], bf16)
            m["wk"] = pack_w(np.asarray(Wk)[sl], bf16)
            m["bq"] = np.asarray(bq, dtype=np.float32)[sl].reshape(128, 1)
            m["bk"] = np.asarray(bk, dtype=np.float32)[sl].reshape(128, 1)
        in_maps.append(m)
    return in_maps


def kernel(x, Wq, bq, Wk, bk, Wv, bv, Wo, bo):
    nc = _get_compiled()
    in_maps = _prep_inputs(x, Wq, bq, Wk, bk, Wv, bv, Wo, bo)
    res = run_bass_kernel_spmd(nc, in_maps, core_ids=list(range(N_CORES)))
    acc = np.zeros((NT, D), dtype=np.float32)
    for c in range(N_CORES):
        acc += np.asarray(res.results[c]["out16"]).astype(np.float32)
    acc += np.asarray(bo, dtype=np.float32)[None, :]
    return acc.reshape(B, S, D)


# revision 46
# speedup vs baseline: 1.0963x; 1.0070x over previous
"""MultiHeadAttention Trainium2 kernel, 8-way tensor-parallel by head.

Problem: B=4, S=2048, D=1024, 16 heads, d_k=64 (nn_MultiHeadAttention_67585605370071).

Sharding: each core owns 2 heads (128 of the 1024 hidden dims):
  Wq/Wk/Wv column-sharded by head, Wo row-sharded; the 8 partial outputs
  are summed on the host (the row-shard reduction) and bo is added there.

Dataflow per core (matmuls bf16 / fp8-DoubleRow, f32 PSUM accumulation):
  - host passes x pre-transposed (xT bf16 [1024, 8192]); Q/K/V projections
    run bf16 (QK_DR=True switches Q/K to fp8 DoubleRow for ~4us, at the
    cost of rel-err 1.5e-2 -> 2.0e-2)
  - V rotated to token-major layout via SBUF->SBUF DMA transposes, then
    quantized fp8 into the augmented layout [V_A | 1 | pad | V_B | 1 | pad]
    by the GpSimd engine; the ones column makes the AV matmul emit softmax
    denominators as row 64 of its PSUM tile
  - scoresT = Kt.T @ Qt per [128,512] half-tile; exp is split between the
    ScalarE ACT (table exp) and a custom DVE microcode op EXP16_POLY_ANT
    ((1 + ks + (ks)^2/2)^16, 8/8 uop stages) so neither engine saturates;
    both emit fp8
  - AV via fp8 DoubleRow matmuls (2 k-tiles per pass), trailing the
    scores/exp stream by TRAIL chunks so the PE never stalls on exp latency
  - normalize via DVE reciprocal + GpSimd partition_broadcast + DVE
    tensor_mul (no PE broadcast matmul)
  - out-proj per 128-token tile; PSUM->SBUF copies alternate between an ACT
    Copy and a DVE tensor_copy; projection bias-adds run as ACT
    Identity+bias (exp/identity/copy share one ACT table - no reloads)
  - emission is a continuous chunk stream: unit u's trailing AVs and
    normalize ride in unit u+1's first chunks, projection closures drain
    with ceiling pacing (fully inside their unit - the next unit's scores
    and trailing AVs depend on kt/vaug), and out-proj matmul/copy pairs
    drain through paced producer/consumer queues that keep the single
    po PSUM bank's write-after-read order identical to emission order
PSUM budget (8 banks): 4x score [128,512] + 2x AV accumulators [65,512]
  + 1 projection bank + 1 out-proj bank.
"""
import numpy as np
import ml_dtypes

import concourse.bass as bass
import concourse.bacc as bacc
import concourse.mybir as mybir
import concourse.tile as tile
from concourse.bass_utils import run_bass_kernel_spmd

import concourse.dve_ops as dve_ops_mod
from concourse.dve_spec import C0, C1, One, Spec, Src0, lower as dve_lower, sq
from concourse.dve_uop import DveOpSpec

BF16 = mybir.dt.bfloat16
F32 = mybir.dt.float32
F8 = mybir.dt.float8e4
bf16 = ml_dtypes.bfloat16
f8 = ml_dtypes.float8_e4m3

B, S, D = 4, 2048, 1024
NT = B * S            # 8192 tokens
N_CORES = 8
KT_PER_B = S // 128   # 16 k-tiles per batch
QB_PER_B = S // 512   # 4 query blocks per batch
TB_PER_B = S // 512   # 4 token blocks per batch (projection)
NCH = KT_PER_B // 2   # 8 score/exp chunks per attention unit

# fp8-e4m3 attention-weights path: expT and V quantized to fp8 so the AV
# matmul can use DoubleRow (2x PE throughput).
VG = 144              # vaug per-k-tile column group
HOFF = 72             # head B column offset inside a group

# Q/K projections in fp8 DoubleRow; weights scaled by 32 on the host, the
# 32*32 factor folded into the exp scale.
QK_DR = False
QK_W_SCALE = 32.0
EXP_SCALE = 0.125 / (QK_W_SCALE * QK_W_SCALE if QK_DR else 1.0)

# exp tiles handed to the DVE custom op instead of ScalarE, per unit
DVE_EXP = {(c, 1) for c in range(7)}
DVE_EXP_LAST = DVE_EXP  # last-batch override (no proj work to hide latency)
VBIAS_DVE = True
SPLIT_EXP = True     # exp per [128,512] half (4 sp bufs) vs full [128,1024]
SPLIT_OUTPROJ = False  # out-proj mm/copy as separate bg closures
NDRAIN = 3           # bg closures drained per chunk
SP_BUFS = 4          # score-PSUM buffers ([128,512] each)
TRAIL_G = 5          # chunks the AV stream trails scores/exp by
MC_POPS = 2          # out-proj producer/consumer pops per chunk
SH_BUFS = 2          # shared pp/po PSUM buffers

_COMPILED = None

# ---------------------------------------------------------------------------
# Custom DVE exp op: out = exp(in * scale) via (1 + ks + (ks)^2/2)^16,
# k = scale/16 — quadratic (4 ALU stages) + 4 squarings = 8/8 v3 stages.
# Rel err vs true exp: rms ~1e-4; after fp8 output quantization 99.96% of
# outputs are bit-identical to fp8(exp(x)).
EXP16_NAME = "EXP16_POLY_ANT"


def _exp16_consts(scale):
    k = scale / 16.0
    return (k * k / 2.0, k)


def _exp16_ref(in0, in1, s0, s1, imm2):
    b = ((in0.astype(np.float32) * s0 + s1) * in0 + 1.0).astype(np.float32)
    for _ in range(4):
        b = (b * b).astype(np.float32)
    return b


def _register_exp16():
    for op in dve_ops_mod.OPS:
        if op.name == EXP16_NAME:
            return op
    body = sq(sq(sq(sq((Src0 * C0 + C1) * Src0 + One))))
    spec = Spec(body=body, reference=_exp16_ref)
    row = max(dve_ops_mod._SUB_OPCODE_FOR_NAME.values()) + 1
    assert row < 0x20, "custom-DVE opcode rows exhausted"
    dve_ops_mod._SUB_OPCODE_FOR_NAME[EXP16_NAME] = row
    shas = {}
    for ver in ("v3", "v4"):
        s = DveOpSpec(name=EXP16_NAME, opcode=row,
                      uops=dve_lower(spec, ver=ver), rd1_en=False)
        shas[ver] = s.sha(ver)
    op = dve_ops_mod.DveOp(EXP16_NAME, spec, False, shas)
    dve_ops_mod.OPS.append(op)
    dve_ops_mod.CUSTOM_DVE_SPECS[EXP16_NAME] = spec
    return op


# ---------------------------------------------------------------------------
def _build():
    exp_op = _register_exp16()
    exp_s0, exp_s1 = _exp16_consts(EXP_SCALE)
    Ident = mybir.ActivationFunctionType.Identity
    CopyF = mybir.ActivationFunctionType.Copy

    nc = bacc.Bacc("TRN2", target_bir_lowering=False, debug=False,
                   num_devices=N_CORES)

    xt_p = nc.declare_dram_parameter("xt", [D, NT], BF16, isOutput=False)
    wv_p = nc.declare_dram_parameter("wv", [128, D], BF16, isOutput=False)
    wo_p = nc.declare_dram_parameter("wo", [128, D], BF16, isOutput=False)
    bq_p = nc.declare_dram_parameter("bq", [128, 1], F32, isOutput=False)
    bk_p = nc.declare_dram_parameter("bk", [128, 1], F32, isOutput=False)
    bv_p = nc.declare_dram_parameter("bv", [128, 1], F32, isOutput=False)
    if QK_DR:
        xt8_p = nc.declare_dram_parameter("xt8", [D, NT], F8, isOutput=False)
        wq_p = nc.declare_dram_parameter("wq", [128, D], F8, isOutput=False)
        wk_p = nc.declare_dram_parameter("wk", [128, D], F8, isOutput=False)
    else:
        wq_p = nc.declare_dram_parameter("wq", [128, D], BF16, isOutput=False)
        wk_p = nc.declare_dram_parameter("wk", [128, D], BF16, isOutput=False)
    out16_p = nc.declare_dram_parameter("out16", [NT, D], BF16, isOutput=True)

    WDT = F8 if QK_DR else BF16

    with tile.TileContext(nc) as tc:
        with (
            tc.tile_pool(name="consts", bufs=1) as consts,
            tc.tile_pool(name="xts", bufs=5) as xts_pool,
            tc.tile_pool(name="qkv", bufs=4) as qkv_pool,
            tc.tile_pool(name="vtrp", bufs=8) as vtr_pool,
            tc.tile_pool(name="expp", bufs=4) as exp_pool,
            tc.tile_pool(name="attnp", bufs=5) as attn_pool,
            tc.tile_pool(name="small", bufs=4) as small,
            tc.tile_pool(name="outp", bufs=5) as out_pool,
            tc.tile_pool(name="ps_sp", bufs=2, space=bass.MemorySpace.PSUM) as ps_sp,
            tc.tile_pool(name="ps_av", bufs=2, space=bass.MemorySpace.PSUM) as ps_av,
            tc.tile_pool(name="ps_pp", bufs=1, space=bass.MemorySpace.PSUM) as ps_pp,
            tc.tile_pool(name="ps_po", bufs=1, space=bass.MemorySpace.PSUM) as ps_po,
        ):
            wq = consts.tile([128, D], WDT, name="wq")
            wk = consts.tile([128, D], WDT, name="wk")
            wv = consts.tile([128, D], BF16, name="wv")
            wo = consts.tile([128, D], BF16, name="wo")
            bq = consts.tile([128, 1], F32, name="bq")
            bk = consts.tile([128, 1], F32, name="bk")
            bv = consts.tile([128, 1], F32, name="bv")
            nc.sync.dma_start(wq[:], wq_p[:])
            nc.sync.dma_start(wk[:], wk_p[:])
            nc.sync.dma_start(wv[:], wv_p[:])
            nc.sync.dma_start(wo[:], wo_p[:])
            nc.sync.dma_start(bq[:], bq_p[:])
            nc.sync.dma_start(bk[:], bk_p[:])
            nc.sync.dma_start(bv[:], bv_p[:])

            # per-batch, per-block persistent tiles, allocated lazily
            qt = {}          # qt[b][tb] -> [128, 512]
            kt = {}          # kt[b][tb] -> [128, 512] (4 k-tiles each)
            vts = {}         # vts[b][tb] -> [128, 512]
            vaug = {}        # vaug[b][tb] -> [128, 4*VG] fp8
            xts_t = {}       # prefetched xT stacks (bf16, fp8)

            def _alloc_batch(b):
                qt[b] = [qkv_pool.tile([128, 512], BF16, tag=f"qt{t}",
                                       name=f"qt{b}_{t}") for t in range(4)]
                kt[b] = [qkv_pool.tile([128, 512], BF16, tag=f"kt{t}",
                                       name=f"kt{b}_{t}") for t in range(4)]
                vts[b] = [qkv_pool.tile([128, 512], BF16, tag=f"vts{t}",
                                        name=f"vts{b}_{t}") for t in range(4)]
                vaug[b] = []
                for t in range(4):
                    va = qkv_pool.tile([128, 4 * VG], F8, tag=f"vaug{t}",
                                       name=f"vaug{b}_{t}")
                    va3 = va.rearrange("p (k c) -> p k c", c=VG)
                    nc.gpsimd.memset(va3[:, :, 64:65], 1.0)
                    nc.gpsimd.memset(va3[:, :, HOFF + 64:HOFF + 65], 1.0)
                    vaug[b].append(va)

            def emit_proj_dma(b, tb):
                """Prefetch the 512-token xT stack(s) for block (b, tb)."""
                if b not in qt:
                    _alloc_batch(b)
                tok0 = b * S + tb * 512
                xts = xts_pool.tile([128, 8 * 512], BF16, tag="xts",
                                    name=f"xts{b}_{tb}")
                src3 = xt_p.rearrange("(kd p) t -> p kd t", p=128)
                dst3 = xts.rearrange("p (kd t) -> p kd t", t=512)
                nc.sync.dma_start(dst3[:, :, :], src3[:, :, tok0:tok0 + 512])
                if QK_DR:
                    xts8 = xts_pool.tile([128, 8 * 512], F8, tag="xts8",
                                         name=f"xts8{b}_{tb}")
                    src83 = xt8_p.rearrange("(kd p) t -> p kd t", p=128)
                    dst83 = xts8.rearrange("p (kd t) -> p kd t", t=512)
                    nc.sync.dma_start(dst83[:, :, :], src83[:, :, tok0:tok0 + 512])
                    xts_t[(b, tb)] = (xts, xts8)
                else:
                    xts_t[(b, tb)] = (xts, xts)

            def emit_proj_compute(b, tb):
                for clo in proj_closures(b, tb):
                    clo()

            def proj_closures(b, tb):
                """The projection block as a list of small closures so its PE
                work can be interleaved between attention score chunks.
                V runs first so its rotation DMAs are in flight long before
                the Pool quantize copies (emitted last) need them."""
                xts, xts8 = xts_t.pop((b, tb))

                # Build producer (PE matmul) and consumer (ACT bias / Pool
                # quantize) closures, then interleave so every consumer
                # drains a few slots after its producer — an in-order
                # engine never head-of-line blocks on a PE matmul.
                ppk = ps_pp.tile([128, 512], F32, tag="pp", name=f"ppk{b}_{tb}")
                ppq = ps_pp.tile([128, 512], F32, tag="pp", name=f"ppq{b}_{tb}")
                ppv = ps_pp.tile([128, 512], F32, tag="pp", name=f"ppv{b}_{tb}")

                def mk_qk(pp, w_sb):
                    if QK_DR:
                        w3 = w_sb.rearrange("p (k m) -> p k m", m=128)
                        x3 = xts8.rearrange("p (k t) -> p k t", t=512)

                        def mmdr(p0):
                            for p in (p0, p0 + 1):
                                nc.tensor.matmul(
                                    pp[:], w3[:, 2 * p:2 * p + 2, :],
                                    x3[:, 2 * p:2 * p + 2, :],
                                    start=(p == 0), stop=(p == 3),
                                    perf_mode=mybir.MatmulPerfMode.DoubleRow)
                        return [lambda p0=p0: mmdr(p0) for p0 in (0, 2)]

                    def mm2(kd0):
                        for kd in (kd0, kd0 + 1):
                            nc.tensor.matmul(
                                pp[:], w_sb[:, 128 * kd:128 * kd + 128],
                                xts[:, 512 * kd:512 * kd + 512],
                                start=(kd == 0), stop=(kd == 7))
                    return [lambda kd0=kd0: mm2(kd0) for kd0 in (0, 2, 4, 6)]

                def mm2v(kd0):
                    for kd in (kd0, kd0 + 1):
                        nc.tensor.matmul(
                            ppv[:], wv[:, 128 * kd:128 * kd + 128],
                            xts[:, 512 * kd:512 * kd + 512],
                            start=(kd == 0), stop=(kd == 7))

                vtrs = [vtr_pool.tile([128, 128], BF16, tag=f"vtr{ti}",
                                      name=f"vtr{b}_{tb}_{ti}")
                        for ti in range(4)]

                def rot_dma():
                    for ti in range(4):
                        nc.sync.dma_start_transpose(
                            vtrs[ti][:], vts[b][tb][:, 128 * ti:128 * ti + 128])

                va4 = vaug[b][tb].rearrange("p (k g e) -> p k g e", g=2, e=HOFF)

                def rot_q(ti):
                    vtr3 = vtrs[ti].rearrange("p (g e) -> p g e", e=64)
                    nc.gpsimd.tensor_copy(va4[:, ti, :, 0:64], vtr3[:, :, :])

                kbias = lambda: nc.scalar.activation(kt[b][tb][:], ppk[:],
                                                     Ident, bias=bk[:])
                qbias = lambda: nc.scalar.activation(qt[b][tb][:], ppq[:],
                                                     Ident, bias=bq[:])
                if VBIAS_DVE:
                    vbias = lambda: nc.vector.tensor_scalar_add(
                        vts[b][tb][:], ppv[:], bv[:])
                else:
                    vbias = lambda: nc.scalar.activation(
                        vts[b][tb][:], ppv[:], Ident, bias=bv[:])
                kmm = mk_qk(ppk, wk)
                qmm = mk_qk(ppq, wq)
                vmm = [lambda kd0=kd0: mm2v(kd0) for kd0 in (0, 2, 4, 6)]
                return (kmm + [kbias] + qmm + [qbias] + vmm
                        + [vbias, rot_dma]
                        + [lambda ti=ti: rot_q(ti) for ti in range(4)])

            def outproj_closures(b, qb, attn):
                """Interleaved [mm, mm, copy, mm, copy, ...] so each
                PSUM->SBUF copy drains ~2 slots after its matmul."""
                obs = [out_pool.tile([128, 1024], BF16, tag="ob",
                                     name=f"ob{b}_{qb}_{ti}")
                       for ti in range(4)]
                pos = {}

                def op_mm(ti, j):
                    po = ps_po.tile([128, 512], F32, tag="po",
                                    name=f"po{b}_{qb}_{ti}_{j}")
                    nc.tensor.matmul(po[:],
                                     attn[:, 128 * ti:128 * ti + 128],
                                     wo[:, 512 * j:512 * j + 512],
                                     start=True, stop=True)
                    pos[(ti, j)] = po

                def op_cp(ti, j, on_act):
                    po = pos.pop((ti, j))
                    ob = obs[ti]
                    if on_act:
                        nc.scalar.activation(ob[:, 512 * j:512 * j + 512],
                                             po[:], CopyF)
                    else:
                        nc.vector.tensor_copy(ob[:, 512 * j:512 * j + 512],
                                              po[:])
                    if j == 1:
                        row0 = b * S + 512 * qb + 128 * ti
                        nc.sync.dma_start(out16_p[row0:row0 + 128, :], ob[:])

                halves = [(ti, j) for ti in range(4) for j in range(2)]
                mms = [lambda ti=ti, j=j: op_mm(ti, j) for ti, j in halves]
                cps = [lambda ti=ti, j=j, on_act=((ti + j) % 2 == 0):
                       op_cp(ti, j, on_act) for ti, j in halves]
                return mms, cps

            TRAIL = TRAIL_G

            class Unit:
                """Per-unit attention state for the continuous chunk stream."""

                def __init__(self, b, qb):
                    self.b, self.qb = b, qb
                    self.av = None
                    self.ecs = {}

                def emit_scores_exp(self, c):
                    b, qb = self.b, self.qb
                    qsl = qt[b][qb]
                    tb_of_c = c // 2        # source projection block
                    cc = c % 2              # k-tile pair within block
                    for h in range(2):
                        ec = exp_pool.tile([128, 1024], F8, tag=f"expt{h}",
                                           bufs=TRAIL_G + 4,
                                           name=f"ec{b}_{qb}_{c}_{h}")
                        if SPLIT_EXP:
                            for j in range(2):
                                sp = ps_sp.tile([128, 512], F32, tag="sp",
                                                bufs=SP_BUFS,
                                                name=f"sp{b}_{qb}_{c}_{h}_{j}")
                                kt_loc = 2 * cc + j
                                nc.tensor.matmul(
                                    sp[:],
                                    kt[b][tb_of_c][64 * h:64 * h + 64,
                                                   128 * kt_loc:128 * kt_loc + 128],
                                    qsl[64 * h:64 * h + 64, :],
                                    start=True, stop=True)
                                ech = ec[:, 512 * j:512 * j + 512]
                                if (c, h) in (DVE_EXP_LAST if b == B - 1 else DVE_EXP):
                                    nc.vector._custom_dve(exp_op, out=ech,
                                                          in0=sp[:],
                                                          s0=exp_s0, s1=exp_s1)
                                else:
                                    nc.scalar.activation(
                                        ech, sp[:],
                                        mybir.ActivationFunctionType.Exp,
                                        scale=EXP_SCALE)
                        else:
                            sp = ps_sp.tile([128, 1024], F32, tag="sp", bufs=2,
                                            name=f"sp{b}_{qb}_{c}_{h}")
                            for j in range(2):
                                kt_loc = 2 * cc + j
                                nc.tensor.matmul(
                                    sp[:, 512 * j:512 * j + 512],
                                    kt[b][tb_of_c][64 * h:64 * h + 64,
                                                   128 * kt_loc:128 * kt_loc + 128],
                                    qsl[64 * h:64 * h + 64, :],
                                    start=True, stop=True)
                            if (c, h) in (DVE_EXP_LAST if b == B - 1 else DVE_EXP):
                                nc.vector._custom_dve(exp_op, out=ec[:],
                                                      in0=sp[:],
                                                      s0=exp_s0, s1=exp_s1)
                            else:
                                nc.scalar.activation(
                                    ec[:], sp[:],
                                    mybir.ActivationFunctionType.Exp,
                                    scale=EXP_SCALE)
                        self.ecs[(c, h)] = ec

                def emit_av(self, cp):
                    b, qb = self.b, self.qb
                    if self.av is None:
                        self.av = [ps_av.tile([65, 512], F32, tag="av",
                                              name=f"av{b}_{qb}_{h}")
                                   for h in range(2)]
                    tb_p = cp // 2
                    ccp = cp % 2
                    va3 = vaug[b][tb_p].rearrange("p (k c) -> p k c", c=VG)
                    for h in range(2):
                        ec3 = self.ecs.pop((cp, h)).rearrange(
                            "p (k q) -> p k q", q=512)
                        nc.tensor.matmul(
                            self.av[h][:],
                            va3[:, 2 * ccp:2 * ccp + 2, HOFF * h:HOFF * h + 65],
                            ec3[:, :, :],
                            start=(cp == 0), stop=(cp == NCH - 1),
                            perf_mode=mybir.MatmulPerfMode.DoubleRow)

                def emit_normalize(self):
                    b, qb = self.b, self.qb
                    attn = attn_pool.tile([128, 512], BF16, tag="attn",
                                          name=f"attn{b}_{qb}")
                    rrows, bcs = [], []
                    for h in range(2):
                        rrow = small.tile([1, 512], F32, tag=f"rrow{h}",
                                          name=f"rr{b}_{qb}_{h}")
                        nc.vector.reciprocal(rrow[:], self.av[h][64:65, :])
                        rrows.append(rrow)
                    for h in range(2):
                        bc_sb = small.tile([64, 512], F32, tag=f"bc_sb{h}",
                                           name=f"bs{b}_{qb}_{h}")
                        nc.gpsimd.partition_broadcast(bc_sb[:], rrows[h])
                        bcs.append(bc_sb)
                    for h in range(2):
                        nc.vector.tensor_mul(attn[64 * h:64 * h + 64, :],
                                             self.av[h][0:64, :], bcs[h])
                    return attn

            # Continuous chunk stream: unit u's chunks 0..7 emit scores+exp;
            # its AV matmuls trail by TRAIL chunks, spilling into unit u+1's
            # first TRAIL chunks; normalize for unit u is emitted right after
            # its last AV (chunk TRAIL of unit u+1), and its out-proj drains
            # later in that unit via the bg queue. Projection blocks for
            # batch b+1 interleave throughout. No engine ever runs dry at a
            # unit boundary.
            from collections import deque
            bgA = deque()        # projection closures (internally ordered)
            bgM = deque()        # out-proj matmuls (producers)
            bgC = deque()        # out-proj PSUM->SBUF copies (consumers)
            nM = nC = 0          # popped counts for producer/consumer pacing
            units = [(b, qb) for b in range(B) for qb in range(QB_PER_B)]

            def drain_mc():
                nonlocal nM, nC
                # po pool is single-buffered: the copy of matmul i must be
                # emitted before matmul i+1 (WAR on the recycled bank)
                if bgC and (nM - nC >= 1 or not bgM):
                    bgC.popleft()()
                    nC += 1
                elif bgM:
                    bgM.popleft()()
                    nM += 1

            for tb in range(TB_PER_B):
                emit_proj_dma(0, tb)
            emit_proj_compute(0, 0)
            bgA.extend(proj_closures(0, 1))
            bgA.extend(proj_closures(0, 2))
            bgA.extend(proj_closures(0, 3))

            prev = None          # unit with AV trail / normalize outstanding
            pend_out = None      # out-proj closures awaiting queue insertion
            for u, (b, qb) in enumerate(units):
                cur = Unit(b, qb)
                carry = 0
                if b + 1 < B:
                    emit_proj_dma(b + 1, qb)
                    newc = proj_closures(b + 1, qb)
                    bgA.extend(newc)
                    # the block added this unit may spill into the next unit:
                    # its kt/vaug are first read >=1 unit later
                    carry = len(newc)
                if pend_out is not None:
                    bgM.extend(pend_out[0])
                    bgC.extend(pend_out[1])
                    pend_out = None
                for c in range(NCH):
                    cur.emit_scores_exp(c)
                    if c >= TRAIL:
                        cur.emit_av(c - TRAIL)
                    elif prev is not None:
                        prev.emit_av(NCH - TRAIL + c)
                        if c == TRAIL - 1:
                            attn = prev.emit_normalize()
                            pend_out = outproj_closures(prev.b, prev.qb, attn)
                    # pace the projection queue so it fully drains within
                    # this unit: chunk c of NCH has (NCH - c) chunks left,
                    # and the next unit's trailing AVs / scores depend on
                    # this unit's projection outputs (kt, vaug)
                    na = max(2, -(-max(0, len(bgA) - carry) // (NCH - c)))
                    for _ in range(na):
                        if bgA:
                            bgA.popleft()()
                    for _ in range(MC_POPS):
                        drain_mc()
                prev = cur
            for c in range(TRAIL):
                prev.emit_av(NCH - TRAIL + c)
            attn = prev.emit_normalize()
            if pend_out is not None:
                bgM.extend(pend_out[0])
                bgC.extend(pend_out[1])
            mo, co = outproj_closures(prev.b, prev.qb, attn)
            bgM.extend(mo)
            bgC.extend(co)
            while bgA or bgM or bgC:
                if bgA:
                    bgA.popleft()()
                drain_mc()

    nc.compile()
    return nc


def _get_compiled():
    global _COMPILED
    if _COMPILED is None:
        _COMPILED = _build()
    return _COMPILED


def _prep_inputs(x, Wq, bq, Wk, bk, Wv, bv, Wo, bo):
    xf = np.asarray(x, dtype=np.float32).reshape(NT, D).T
    xt = np.ascontiguousarray(xf).astype(bf16)

    def pack_w(Wc, dtype, scale=1.0):
        # [128 out, 1024 in] -> k-tile packed [128, 1024]
        wt = np.asarray(Wc, dtype=np.float32).T * scale  # [1024 in, 128 out]
        return np.ascontiguousarray(
            wt.reshape(8, 128, 128).transpose(1, 0, 2).reshape(128, D)).astype(dtype)

    if QK_DR:
        xt8 = np.ascontiguousarray(xf).astype(f8)

    in_maps = []
    for c in range(N_CORES):
        sl = slice(128 * c, 128 * c + 128)
        m = {
            "xt": xt,
            "wv": pack_w(np.asarray(Wv)[sl], bf16),
            "wo": np.ascontiguousarray(
                np.asarray(Wo, dtype=np.float32)[:, sl].T).astype(bf16),
            "bv": np.asarray(bv, dtype=np.float32)[sl].reshape(128, 1),
        }
        if QK_DR:
            m["xt8"] = xt8
            m["wq"] = pack_w(np.asarray(Wq)[sl], f8, QK_W_SCALE)
            m["wk"] = pack_w(np.asarray(Wk)[sl], f8, QK_W_SCALE)
            m["bq"] = (np.asarray(bq, dtype=np.float32)[sl] *
                       QK_W_SCALE).reshape(128, 1)
            m["bk"] = (np.asarray(bk, dtype=np.float32)[sl] *
                       QK_W_SCALE).reshape(128, 1)
        else:
            m["wq"] = pack_w(np.asarray(Wq)[sl], bf16)
            m["wk"] = pack_w(np.asarray(Wk)[sl], bf16)
            m["bq"] = np.asarray(bq, dtype=np.float32)[sl].reshape(128, 1)
            m["bk"] = np.asarray(bk, dtype=np.float32)[sl].reshape(128, 1)
        in_maps.append(m)
    return in_maps


def kernel(x, Wq, bq, Wk, bk, Wv, bv, Wo, bo):
    nc = _get_compiled()
    in_maps = _prep_inputs(x, Wq, bq, Wk, bk, Wv, bv, Wo, bo)
    res = run_bass_kernel_spmd(nc, in_maps, core_ids=list(range(N_CORES)))
    acc = np.zeros((NT, D), dtype=np.float32)
    for c in range(N_CORES):
        acc += np.asarray(res.results[c]["out16"]).astype(np.float32)
    acc += np.asarray(bo, dtype=np.float32)[None, :]
    return acc.reshape(B, S, D)
